# revision 23
# baseline (speedup 1.0000x reference)
"""Trainium2 Bass kernel for nn_AttentionConv (sparse checkerboard attention).

Math (per batch image, C=64, H=W=32, N=4096 upsampled tokens):
  q,k,v = 1x1 convs; q is bilinearly 2x-upsampled, k/v zero-upsampled
  (values only at (even,even) positions).  A checkerboard mask of -1e8 is
  added to k itself, so the 3072 masked key columns are all identically
  (-1e8,...,-1e8): their score for query n is -1e8*S(n) with
  S(n)=sum_d q_up[n,d], and their v is 0.  Hence
     out[c,n] = sum_{m' in 1024 unmasked} v[c,m'] exp(s[n,m']) / D(n)
     D(n)     = 3072*exp(-1e8*S(n)) + sum_{m'} exp(s[n,m'])
  with s[n,m'] = q_up[n,:].k[:,m'].

All exps carry a constant bias of -20 (out = num'/den' is invariant):
den' = den*e^-20 then spans [2e-6, 7e7], inside the domain where the ACT
ln table is accurate (it breaks past ~2^+-64), so 1/den can run on the
ACT engine as exp(-ln(den)) -- the DVE reciprocal is ~8 cycles/element
on a single lane and the ACT Reciprocal function is blocked in bass.

Device pipeline per core (matmul operands f32r; the big v^T.p
accumulation runs in bf16; PSUM accumulates in f32):
  k    = Wk x                                            [8, 1024]
  vT   = x^T Wv^T per 128-token chunk -> vTa (bf16, +ones denom row)
  t2   = kron(Wq, Ah_slice) contraction over the <=10 source rows the
         16-row slice touches (5 chunks of 128)          [32, 128]
  qfT  = col-interp of t2 via awT                        [8, 1024]
  dex  = exp(-1e8*max(S,-4.65e-7) - 20): masked-key denominator term;
         the clamp keeps the S<0 saturation finite (~1e15) because the
         exp table emits NaN for huge positive args
  loop over 8 key tiles: sT = k_t^T qfT -> exp -> bf16 pT ->
         out_ps[65,1024] += [v_t;1]^T pT
  rden = exp(-ln(den)), broadcast on GPSIMD, multiply on DVE, DMA out.

Sharding: 8 cores = 2 batches x 4 query-slices of 1024 tokens
(16 upsampled rows each).  No collectives; each core writes a disjoint
[64, 1024] output slice.
"""
import sys

import numpy as np

if "/opt/trn_rl_repo" not in sys.path:
    sys.path.insert(0, "/opt/trn_rl_repo")

B, C, H, W = 2, 64, 32, 32
D = 8          # q/k head dim
NQ = 1024      # query tokens per core (16 upsampled rows x 64 cols)
NK = 1024      # unmasked keys per image (= H*W)
N_CORES = 8
RWIN = 10      # source rows touched by one 16-row upsampled slice
NCHUNK = (C * RWIN) // 128  # = 5 kron contraction chunks
R_START = (0, 7, 15, 22)    # first source row per slice


def _lin_interp_mat(n_in, n_out):
    # float32 replica of reference's bilinear (align_corners=True) matrix
    pos = np.arange(n_out, dtype=np.float32) * np.float32(
        (n_in - 1) / (n_out - 1)
    )
    i0 = np.clip(np.floor(pos), 0, n_in - 2).astype(np.int32)
    w = (pos - i0.astype(np.float32)).astype(np.float32)
    A = np.zeros((n_out, n_in), np.float32)
    r = np.arange(n_out)
    np.add.at(A, (r, i0), 1.0 - w)
    np.add.at(A, (r, i0 + 1), w)
    return A


def _patched_act_tables(orig):
    """Pin Exp/Ln/Copy to the one table set that holds all of them
    (natural_log_exp_and_others).  The greedy table-load pass otherwise
    ping-pongs between an exp-only and an ln-only set in the epilogue,
    costing 4 extra 1.3us ACT_TABLE_LOADs on the critical path."""
    import functools

    @functools.cache
    def patched(arch):
        tabs = dict(orig(arch))
        combined = None
        for name, s in tabs.items():
            names = {f.name for f in s}
            if {"Exp", "Ln", "Copy"} <= names:
                combined = name
                break
        if combined is None:
            return tabs
        keep = tabs[combined]
        return {
            name: (s if name == combined else s - keep)
            for name, s in tabs.items()
        }

    return patched


def _build_nc():
    import concourse.bacc as bacc
    import concourse.mybir as mybir
    from concourse import tile

    f32 = mybir.dt.float32
    f32r = mybir.dt.float32r
    bf16 = mybir.dt.bfloat16
    EXP = mybir.ActivationFunctionType.Exp
    LN = mybir.ActivationFunctionType.Ln

    orig_tables = bacc.get_activation_tables
    bacc.get_activation_tables = _patched_act_tables(orig_tables)
    try:
        return _build_nc_inner(bacc, mybir, tile, f32, f32r, bf16, EXP, LN)
    finally:
        bacc.get_activation_tables = orig_tables


def _build_nc_inner(bacc, mybir, tile, f32, f32r, bf16, EXP, LN):
    nc = bacc.Bacc(None, target_bir_lowering=False)

    xb_e = nc.declare_dram_parameter("xb", [C, H * W], f32r, isOutput=False)
    xchr_e = nc.declare_dram_parameter(
        "xchr", [128, NCHUNK * W], f32r, isOutput=False
    )
    gm_e = nc.declare_dram_parameter(
        "gmat", [128, NCHUNK * 128], f32r, isOutput=False
    )
    wp_e = nc.declare_dram_parameter("wpack", [C, 136], f32r, isOutput=False)
    out_e = nc.declare_dram_parameter("out", [C, NQ], f32, isOutput=True)

    hA = slice(0, 512)
    hB = slice(512, 1024)

    with tile.TileContext(nc) as tc:
        with (
            nc.allow_low_precision(
                reason="f32r/bf16 matmul operands; PSUM accumulates in f32"
            ),
            tc.tile_pool(name="const", bufs=1) as cst,
            tc.tile_pool(name="sb", bufs=1) as sbp,
            tc.tile_pool(name="pexp", bufs=3) as pexp,
        ):
            # ---- input DMAs.  Each queue pays ~2.4us startup latency on
            # its first transfer then streams at ~200GB/s, so xb halves
            # lead the scalar/gpsimd queues and everything lands ~11us.
            xb = cst.tile([C, H * W], f32r)
            nc.scalar.dma_start(xb[:, hA], xb_e[:, hA])
            nc.gpsimd.dma_start(xb[:, hB], xb_e[:, hB])
            wpack = cst.tile([C, 136], f32r)
            nc.sync.dma_start(wpack[:], wp_e[:])
            xchr = cst.tile([128, NCHUNK * W], f32r)
            nc.sync.dma_start(xchr[:], xchr_e[:])
            gm = cst.tile([128, NCHUNK * 128], f32r)
            nc.scalar.dma_start(gm[:, 0 : 2 * 128], gm_e[:, 0 : 2 * 128])
            nc.gpsimd.dma_start(gm[:, 2 * 128 : 4 * 128], gm_e[:, 2 * 128 : 4 * 128])
            nc.sync.dma_start(gm[:, 4 * 128 : 5 * 128], gm_e[:, 4 * 128 : 5 * 128])
            wk = wpack[:, 0:D]
            wv = wpack[:, D : D + C]
            awT = wpack[0:W, D + C : D + C + 64]

            # ---- small constants (memset f32, cast to f32r) ----
            cf = cst.tile([D, 1], f32)
            nc.vector.memset(cf[:], 1.0)
            ones8 = cst.tile([D, 1], f32r)
            nc.vector.tensor_copy(ones8[:], cf[:])
            ones64f = cst.tile([1, C], f32)
            nc.vector.memset(ones64f[:], 1.0)
            ones64 = cst.tile([1, C], f32r)
            nc.vector.tensor_copy(ones64[:], ones64f[:])
            e65f = cst.tile([1, C + 1], f32)
            nc.vector.memset(e65f[:], 0.0)
            nc.vector.memset(e65f[:, C : C + 1], 3072.0)
            e65 = cst.tile([1, C + 1], f32r)
            nc.vector.tensor_copy(e65[:], e65f[:])
            zb = cst.tile([1, 1], f32)
            nc.vector.memset(zb[:], 0.0)
            # constant bias -20 on every score exp; see module docstring
            mB = cst.tile([128, 1], f32)
            nc.vector.memset(mB[:], -20.0)

            # ---- persistent SBUF tiles ----
            k_sb = sbp.tile([D, NK], f32r)
            qfT = sbp.tile([D, NQ], f32r)
            dex = sbp.tile([1, NQ], f32r)
            s_cl = sbp.tile([1, NQ], f32)
            vTa = sbp.tile([128, 8 * (C + 1)], bf16)
            nc.vector.memset(
                vTa[:].rearrange("p (t c) -> p t c", c=C + 1)[:, :, C], 1.0
            )

            with tc.tile_pool(name="ps_o", bufs=1, space="PSUM") as pso:
                out_ps = pso.tile([C + 1, NQ], f32)

                # ---- setup matmuls (all f32r) ----
                with tc.tile_pool(name="ps_s", bufs=1, space="PSUM") as pss:
                    # k projection first: only needs xb + wpack.  One
                    # PSUM bank, halves serialized on the copy (off the
                    # critical path); v tiles (double-buffered) fill PE
                    # while the kron operands stream in.
                    def vt_tile(u):
                        vt_ps = pss.tile(
                            [128, C], f32, tag="vtA" if u % 2 == 0 else "vtB"
                        )
                        nc.tensor.matmul(
                            vt_ps[:],
                            xb[:, u * 128 : (u + 1) * 128],
                            wv,
                            start=True,
                            stop=True,
                        )
                        nc.vector.tensor_copy(
                            vTa[:, u * (C + 1) : u * (C + 1) + C], vt_ps[:]
                        )

                    k_psA = pss.tile([D, 512], f32, tag="kps")
                    nc.tensor.matmul(
                        k_psA[:], wk, xb[:, hA], start=True, stop=True
                    )
                    nc.vector.tensor_copy(k_sb[:, hA], k_psA[:])
                    for u in range(3):
                        vt_tile(u)
                    k_psB = pss.tile([D, 512], f32, tag="kps")
                    nc.tensor.matmul(
                        k_psB[:], wk, xb[:, hB], start=True, stop=True
                    )
                    nc.scalar.copy(k_sb[:, hB], k_psB[:])

                    # fused q-proj + row-interp (kron), then col-interp
                    t2_ps = pss.tile([W, 128], f32, tag="t2")
                    for k in range(NCHUNK):
                        nc.tensor.matmul(
                            t2_ps[:],
                            xchr[:, k * W : (k + 1) * W],
                            gm[:, k * 128 : (k + 1) * 128],
                            start=(k == 0),
                            stop=(k == NCHUNK - 1),
                            skip_group_check=True,
                        )
                    t2_sb = sbp.tile([W, 128], f32r)
                    nc.scalar.copy(t2_sb[:], t2_ps[:])
                    for u in range(3, 8):
                        vt_tile(u)

                    # col-interp in two independent banks so each half's
                    # PSUM->SBUF copy starts as soon as its 8 matmuls end
                    qf_psA = pss.tile([D, 512], f32, tag="qfA")
                    qf_psB = pss.tile([D, 512], f32, tag="qfB")
                    t2_v = t2_sb[:].rearrange("c (d i) -> c i d", i=16)
                    for i in range(16):
                        dst = qf_psA if i < 8 else qf_psB
                        nc.tensor.matmul(
                            dst[:, (i % 8) * 64 : (i % 8) * 64 + 64],
                            t2_v[:, i, :],
                            awT,
                            start=True,
                            stop=True,
                        )
                    nc.vector.tensor_copy(qfT[:, hA], qf_psA[:])
                    nc.scalar.copy(qfT[:, hB], qf_psB[:])

                # S = colsum(qfT) via GPSIMD partition all-reduce (keeps
                # the PE free for the score loop), clamped on DVE; dex on
                # ACT later (emitted into the loop, not before exp0)
                from concourse import bass_isa

                s_all = sbp.tile([D, NQ], f32)
                nc.gpsimd.partition_all_reduce(
                    s_all[:], qfT[:], channels=D,
                    reduce_op=bass_isa.ReduceOp.add,
                )
                nc.vector.tensor_scalar(
                    s_cl[:], s_all[0:1, :], -4.65e-7, None,
                    mybir.AluOpType.max,
                )

                # ---- main loop: scores^T, exp, accumulate ----
                with tc.tile_pool(
                    name="ps_sc", bufs=2, space="PSUM"
                ) as pssc:
                    for t in range(8):
                        sT = pssc.tile([128, NQ], f32)
                        kT_t = k_sb[:, t * 128 : (t + 1) * 128]
                        for h in (hA, hB):
                            nc.tensor.matmul(
                                sT[:, h], kT_t, qfT[:, h], start=True,
                                stop=True,
                            )
                        pT = pexp.tile([128, NQ], bf16, tag="pT")
                        if t < 7:
                            nc.scalar.activation(
                                pT[:], sT[:], EXP, bias=mB[:]
                            )
                        else:
                            # halved so out7A (stop) lands earlier and the
                            # A-half epilogue overlaps the B-half tail
                            nc.scalar.activation(
                                pT[:, hA], sT[:, hA], EXP, bias=mB[:]
                            )
                            nc.scalar.activation(
                                pT[:, hB], sT[:, hB], EXP, bias=mB[:]
                            )
                        if t == 5:
                            # masked-key denominator term, off the
                            # critical path
                            nc.scalar.activation(
                                dex[:], s_cl[:], EXP, bias=mB[0:1, :],
                                scale=-1.0e8,
                            )
                        for h in (hA, hB):
                            nc.tensor.matmul(
                                out_ps[:, h],
                                vTa[:, t * (C + 1) : (t + 1) * (C + 1)],
                                pT[:, h],
                                start=(t == 0),
                                stop=(t == 7),
                                skip_group_check=True,
                            )
                        if t == 5:
                            for h in (hA, hB):
                                nc.tensor.matmul(
                                    out_ps[:, h],
                                    e65[:],
                                    dex[:, h],
                                    start=False,
                                    stop=False,
                                    skip_group_check=True,
                                )

                # ---- epilogue: rden = exp(-ln(den)) on ACT, broadcast
                # on GPSIMD (SBUF->SBUF, so the DVE multiply's only PSUM
                # operand is num), multiply on DVE, DMA out.  Quartered
                # and fully per-quarter tiles so the chain pipelines. ----
                den = out_ps[C : C + 1, :]
                dmas = (nc.sync, nc.scalar, nc.sync, nc.scalar)
                for qi in range(4):
                    q = slice(qi * 256, (qi + 1) * 256)
                    lden = sbp.tile([1, 256], f32, tag=f"ld{qi}")
                    nc.scalar.activation(lden[:], den[:, q], LN, bias=zb[:])
                    rden = sbp.tile([1, 256], f32, tag=f"rd{qi}")
                    nc.scalar.activation(
                        rden[:], lden[:], EXP, bias=zb[:], scale=-1.0
                    )
                    bc_sb = sbp.tile([C, 256], f32, tag=f"bc{qi}")
                    nc.gpsimd.partition_broadcast(bc_sb[:], rden[:])
                    fin = sbp.tile([C, 256], f32, tag=f"fin{qi}")
                    nc.vector.tensor_mul(fin[:], out_ps[0:C, q], bc_sb[:])
                    dmas[qi].dma_start(out_e[:, q], fin[:])

    nc.finalize()
    return nc


_NC = None


def _get_nc():
    global _NC
    if _NC is None:
        _NC = _build_nc()
    return _NC


def _in_maps(x, Wq, Wk, Wv):
    x = np.asarray(x, np.float32)
    Wq = np.asarray(Wq, np.float32)
    Wk = np.asarray(Wk, np.float32)
    Wv = np.asarray(Wv, np.float32)
    Ah = _lin_interp_mat(H, 2 * H)
    awT = np.ascontiguousarray(_lin_interp_mat(W, 2 * W).T)  # [32, 64]
    wpack = np.zeros((C, 136), np.float32)
    wpack[:, 0:D] = Wk.T
    wpack[:, D : D + C] = Wv.T
    wpack[0:W, D + C : D + C + 64] = awT
    # G_s[(c rloc), (d i)] = Wq[d, c] * Ah_s[i, r0+rloc], c-major flat
    # index (c*RWIN + rloc) split into NCHUNK chunks of 128
    gms = []
    for s in range(4):
        r0 = R_START[s]
        Ah_s = Ah[s * 16 : (s + 1) * 16, r0 : r0 + RWIN]  # [16, RWIN]
        G = np.kron(Wq.T, Ah_s.T)  # [C*RWIN, 128] = [640, 128]
        gms.append(
            np.ascontiguousarray(
                G.reshape(NCHUNK, 128, 128)
                .transpose(1, 0, 2)
                .reshape(128, NCHUNK * 128)
            )
        )
    maps = []
    for i in range(N_CORES):
        b, s = divmod(i, 4)
        r0 = R_START[s]
        xb = np.ascontiguousarray(x[b].reshape(C, H * W))
        xsub = x[b][:, r0 : r0 + RWIN, :]  # [C, RWIN, W]
        xchr = np.ascontiguousarray(
            xsub.reshape(NCHUNK, 128, W).transpose(1, 0, 2).reshape(128, -1)
        )
        maps.append({"xb": xb, "xchr": xchr, "gmat": gms[s], "wpack": wpack})
    return maps


def _run(x, Wq, Wk, Wv, trace=False):
    from concourse.bass_utils import run_bass_kernel_spmd

    nc = _get_nc()
    res = run_bass_kernel_spmd(
        nc, _in_maps(x, Wq, Wk, Wv), core_ids=list(range(N_CORES)), trace=trace
    )
    out = np.empty((B, C, 4 * H * W), np.float32)
    for i in range(N_CORES):
        b, s = divmod(i, 4)
        out[b, :, s * NQ : (s + 1) * NQ] = res.results[i]["out"]
    return out.reshape(B, C, 2 * W, 2 * H), res


def kernel(x, Wq, Wk, Wv):
    out, _ = _run(x, Wq, Wk, Wv)
    return out


# revision 40
# speedup vs baseline: 1.1744x; 1.1744x over previous
"""Trainium2 Bass kernel for nn_AttentionConv (sparse checkerboard attention).

Math (per batch image, C=64, H=W=32, N=4096 upsampled tokens):
  q,k,v = 1x1 convs; q is bilinearly 2x-upsampled, k/v zero-upsampled
  (values only at (even,even) positions).  A checkerboard mask of -1e8 is
  added to k itself, so the 3072 masked key columns are all identically
  (-1e8,...,-1e8): their score for query n is -1e8*S(n) with
  S(n)=sum_d q_up[n,d], and their v is 0.  Hence
     out[c,n] = sum_{m' in 1024 unmasked} v[c,m'] exp(s[n,m']) / D(n)
     D(n)     = 3072*exp(-1e8*S(n)) + sum_{m'} exp(s[n,m'])
  with s[n,m'] = q_up[n,:].k[:,m'].

All exps carry a constant bias of -20 (out = num'/den' is invariant):
den' = den*e^-20 then spans [2e-6, 7e7], inside the domain where the ACT
ln table is accurate (it breaks past ~2^+-64), so 1/den can run on the
ACT engine as exp(-ln(den)) -- the DVE reciprocal is ~8 cycles/element
on a single lane and the ACT Reciprocal function is blocked in bass.

Device pipeline per core (matmul operands f32r; the big v^T.p
accumulation runs in bf16; PSUM accumulates in f32):
  k    = Wk x                                            [8, 1024]
  vT   = x^T Wv^T per 128-token chunk -> vTa (bf16, +ones denom row)
  t2   = kron(Wq, Ah_slice) contraction over the <=10 source rows the
         16-row slice touches (5 chunks of 128)          [32, 128]
  qfT  = col-interp of t2 via awT                        [8, 1024]
  dex  = exp(-1e8*max(S,-4.65e-7) - 20): masked-key denominator term;
         the clamp keeps the S<0 saturation finite (~1e15) because the
         exp table emits NaN for huge positive args
  loop over 8 key tiles: sT = k_t^T qfT -> exp -> bf16 pT ->
         out_ps[65,1024] += [v_t;1]^T pT
  rden = exp(-ln(den)), broadcast on GPSIMD, multiply on DVE, DMA out.

Sharding: 8 cores = 2 batches x 4 query-slices of 1024 tokens
(16 upsampled rows each).  No collectives; each core writes a disjoint
[64, 1024] output slice.
"""
import sys

import numpy as np

if "/opt/trn_rl_repo" not in sys.path:
    sys.path.insert(0, "/opt/trn_rl_repo")

B, C, H, W = 2, 64, 32, 32
D = 8          # q/k head dim
NQ = 1024      # query tokens per core (16 upsampled rows x 64 cols)
NK = 1024      # unmasked keys per image (= H*W)
N_CORES = 8
RWIN = 10      # source rows touched by one 16-row upsampled slice
NCHUNK = (C * RWIN) // 128  # = 5 kron contraction chunks
R_START = (0, 7, 15, 22)    # first source row per slice


def _lin_interp_mat(n_in, n_out):
    # float32 replica of reference's bilinear (align_corners=True) matrix
    pos = np.arange(n_out, dtype=np.float32) * np.float32(
        (n_in - 1) / (n_out - 1)
    )
    i0 = np.clip(np.floor(pos), 0, n_in - 2).astype(np.int32)
    w = (pos - i0.astype(np.float32)).astype(np.float32)
    A = np.zeros((n_out, n_in), np.float32)
    r = np.arange(n_out)
    np.add.at(A, (r, i0), 1.0 - w)
    np.add.at(A, (r, i0 + 1), w)
    return A


def _patched_act_tables(orig):
    """Pin Exp/Ln/Copy to the one table set that holds all of them
    (natural_log_exp_and_others).  The greedy table-load pass otherwise
    ping-pongs between an exp-only and an ln-only set in the epilogue,
    costing 4 extra 1.3us ACT_TABLE_LOADs on the critical path."""
    import functools

    @functools.cache
    def patched(arch):
        tabs = dict(orig(arch))
        combined = None
        for name, s in tabs.items():
            names = {f.name for f in s}
            if {"Exp", "Ln", "Copy"} <= names:
                combined = name
                break
        if combined is None:
            return tabs
        keep = tabs[combined]
        return {
            name: (s if name == combined else s - keep)
            for name, s in tabs.items()
        }

    return patched


def _build_nc():
    import concourse.bacc as bacc
    import concourse.mybir as mybir
    from concourse import tile

    f32 = mybir.dt.float32
    f32r = mybir.dt.float32r
    bf16 = mybir.dt.bfloat16
    EXP = mybir.ActivationFunctionType.Exp
    LN = mybir.ActivationFunctionType.Ln

    orig_tables = bacc.get_activation_tables
    bacc.get_activation_tables = _patched_act_tables(orig_tables)
    try:
        return _build_nc_inner(bacc, mybir, tile, f32, f32r, bf16, EXP, LN)
    finally:
        bacc.get_activation_tables = orig_tables


def _build_nc_inner(bacc, mybir, tile, f32, f32r, bf16, EXP, LN):
    nc = bacc.Bacc(None, target_bir_lowering=False)

    xb_e = nc.declare_dram_parameter("xb", [C, H * W], f32r, isOutput=False)
    xchr_e = nc.declare_dram_parameter(
        "xchr", [128, NCHUNK * W], f32r, isOutput=False
    )
    gm_e = nc.declare_dram_parameter(
        "gmat", [128, NCHUNK * 128], f32r, isOutput=False
    )
    wp_e = nc.declare_dram_parameter("wpack", [C, 136], f32r, isOutput=False)
    out_e = nc.declare_dram_parameter("out", [C, NQ], f32, isOutput=True)

    hA = slice(0, 512)
    hB = slice(512, 1024)

    with tile.TileContext(nc) as tc:
        with (
            nc.allow_low_precision(
                reason="f32r/bf16 matmul operands; PSUM accumulates in f32"
            ),
            tc.tile_pool(name="const", bufs=1) as cst,
            tc.tile_pool(name="sb", bufs=1) as sbp,
            tc.tile_pool(name="pexp", bufs=3) as pexp,
        ):
            # ---- input DMAs.  Each queue pays ~2.4us startup latency on
            # its first transfer then streams at ~200GB/s, so xb halves
            # lead the scalar/gpsimd queues and everything lands ~11us.
            xb = cst.tile([C, H * W], f32r)
            nc.scalar.dma_start(xb[:, hA], xb_e[:, hA])
            nc.gpsimd.dma_start(xb[:, hB], xb_e[:, hB])
            wpack = cst.tile([C, 136], f32r)
            nc.sync.dma_start(wpack[:], wp_e[:])
            xchr = cst.tile([128, NCHUNK * W], f32r)
            nc.sync.dma_start(xchr[:], xchr_e[:])
            gm = cst.tile([128, NCHUNK * 128], f32r)
            nc.scalar.dma_start(gm[:, 0 : 2 * 128], gm_e[:, 0 : 2 * 128])
            nc.gpsimd.dma_start(gm[:, 2 * 128 : 4 * 128], gm_e[:, 2 * 128 : 4 * 128])
            nc.sync.dma_start(gm[:, 4 * 128 : 5 * 128], gm_e[:, 4 * 128 : 5 * 128])
            wk = wpack[:, 0:D]
            wv = wpack[:, D : D + C]
            awT = wpack[0:W, D + C : D + C + 64]

            # ---- small constants (memset f32, cast to f32r) ----
            cf = cst.tile([D, 1], f32)
            nc.vector.memset(cf[:], 1.0)
            ones8 = cst.tile([D, 1], f32r)
            nc.vector.tensor_copy(ones8[:], cf[:])
            e65f = cst.tile([1, C + 1], f32)
            nc.vector.memset(e65f[:], 0.0)
            nc.vector.memset(e65f[:, C : C + 1], 3072.0)
            e65 = cst.tile([1, C + 1], f32r)
            nc.vector.tensor_copy(e65[:], e65f[:])
            zb = cst.tile([1, 1], f32)
            nc.vector.memset(zb[:], 0.0)
            # constant bias -20 on every score exp; see module docstring
            mB = cst.tile([128, 1], f32)
            nc.vector.memset(mB[:], -20.0)

            # ---- persistent SBUF tiles ----
            k_sb = sbp.tile([D, NK], f32r)
            qfT = sbp.tile([D, NQ], f32r)
            dex = sbp.tile([1, NQ], f32r)
            s_cl = sbp.tile([1, NQ], f32)
            vTa = sbp.tile([128, 8 * (C + 1)], bf16)
            nc.vector.memset(
                vTa[:].rearrange("p (t c) -> p t c", c=C + 1)[:, :, C], 1.0
            )

            with tc.tile_pool(name="ps_o", bufs=1, space="PSUM") as pso:
                out_ps = pso.tile([C + 1, NQ], f32)

                # ---- setup matmuls (all f32r) ----
                with tc.tile_pool(name="ps_s", bufs=1, space="PSUM") as pss:
                    # k projection first: only needs xb + wpack.  One
                    # PSUM bank, halves serialized on the copy (off the
                    # critical path); v tiles (double-buffered) fill PE
                    # while the kron operands stream in.
                    def vt_tile(u):
                        vt_ps = pss.tile(
                            [128, C], f32, tag="vtA" if u % 2 == 0 else "vtB"
                        )
                        nc.tensor.matmul(
                            vt_ps[:],
                            xb[:, u * 128 : (u + 1) * 128],
                            wv,
                            start=True,
                            stop=True,
                        )
                        nc.vector.tensor_copy(
                            vTa[:, u * (C + 1) : u * (C + 1) + C], vt_ps[:]
                        )

                    k_psA = pss.tile([D, 512], f32, tag="kps")
                    nc.tensor.matmul(
                        k_psA[:], wk, xb[:, hA], start=True, stop=True
                    )
                    nc.vector.tensor_copy(k_sb[:, hA], k_psA[:])
                    for u in range(3):
                        vt_tile(u)
                    k_psB = pss.tile([D, 512], f32, tag="kps")
                    nc.tensor.matmul(
                        k_psB[:], wk, xb[:, hB], start=True, stop=True
                    )
                    nc.scalar.copy(k_sb[:, hB], k_psB[:])

                    # fused q-proj + row-interp (kron), then col-interp
                    t2_ps = pss.tile([W, 128], f32, tag="t2")
                    for k in range(NCHUNK):
                        nc.tensor.matmul(
                            t2_ps[:],
                            xchr[:, k * W : (k + 1) * W],
                            gm[:, k * 128 : (k + 1) * 128],
                            start=(k == 0),
                            stop=(k == NCHUNK - 1),
                            skip_group_check=True,
                        )
                    t2_sb = sbp.tile([W, 128], f32r)
                    nc.scalar.copy(t2_sb[:], t2_ps[:])
                    for u in range(3, 8):
                        vt_tile(u)

                    # col-interp in two independent banks so each half's
                    # PSUM->SBUF copy starts as soon as its 8 matmuls end
                    qf_psA = pss.tile([D, 512], f32, tag="qfA")
                    qf_psB = pss.tile([D, 512], f32, tag="qfB")
                    t2_v = t2_sb[:].rearrange("c (i d) -> c i d", i=16)
                    for i in range(16):
                        dst = qf_psA if i < 8 else qf_psB
                        nc.tensor.matmul(
                            dst[:, (i % 8) * 64 : (i % 8) * 64 + 64],
                            t2_v[:, i, :],
                            awT,
                            start=True,
                            stop=True,
                        )
                    nc.vector.tensor_copy(qfT[:, hA], qf_psA[:])
                    nc.scalar.copy(qfT[:, hB], qf_psB[:])


                # ---- main loop: scores^T, exp, accumulate ----
                with tc.tile_pool(
                    name="ps_sc", bufs=2, space="PSUM"
                ) as pssc:
                    for t in range(8):
                        sT = pssc.tile([128, NQ], f32)
                        kT_t = k_sb[:, t * 128 : (t + 1) * 128]
                        for h in (hA, hB):
                            nc.tensor.matmul(
                                sT[:, h], kT_t, qfT[:, h], start=True,
                                stop=True,
                            )
                        if t == 0:
                            # S = colsum(qfT) + clamp, in the exp0 shadow
                            # (PE idles waiting for pT otherwise).  Summing
                            # the same f32r qfT the scores use keeps the
                            # sign of borderline-|S| queries aligned with
                            # the reference's masked-row selection.
                            for h in (hA, hB):
                                s_ps = pssc.tile([1, 512], f32, tag="srow")
                                nc.tensor.matmul(
                                    s_ps[:], ones8[:], qfT[:, h],
                                    start=True, stop=True,
                                )
                                nc.vector.tensor_scalar(
                                    s_cl[:, h], s_ps[:], -4.65e-7, None,
                                    mybir.AluOpType.max,
                                )
                        pT = pexp.tile([128, NQ], bf16, tag="pT")
                        if t < 7:
                            nc.scalar.activation(
                                pT[:], sT[:], EXP, bias=mB[:]
                            )
                        else:
                            # halved so out7A (stop) lands earlier and the
                            # A-half epilogue overlaps the B-half tail
                            nc.scalar.activation(
                                pT[:, hA], sT[:, hA], EXP, bias=mB[:]
                            )
                            nc.scalar.activation(
                                pT[:, hB], sT[:, hB], EXP, bias=mB[:]
                            )
                        if t == 5:
                            # masked-key denominator term, off the
                            # critical path
                            nc.scalar.activation(
                                dex[:], s_cl[:], EXP, bias=mB[0:1, :],
                                scale=-1.0e8,
                            )
                        for h in (hA, hB):
                            nc.tensor.matmul(
                                out_ps[:, h],
                                vTa[:, t * (C + 1) : (t + 1) * (C + 1)],
                                pT[:, h],
                                start=(t == 0),
                                stop=(t == 7),
                                skip_group_check=True,
                            )
                        if t == 5:
                            for h in (hA, hB):
                                nc.tensor.matmul(
                                    out_ps[:, h],
                                    e65[:],
                                    dex[:, h],
                                    start=False,
                                    stop=False,
                                    skip_group_check=True,
                                )

                # ---- epilogue: rden = exp(-ln(den)) on ACT, broadcast
                # on GPSIMD (SBUF->SBUF, so the DVE multiply's only PSUM
                # operand is num), multiply on DVE, DMA out.  Quartered
                # and fully per-quarter tiles so the chain pipelines. ----
                den = out_ps[C : C + 1, :]
                dmas = (nc.sync, nc.scalar, nc.sync, nc.scalar)
                for qi in range(4):
                    q = slice(qi * 256, (qi + 1) * 256)
                    lden = sbp.tile([1, 256], f32, tag=f"ld{qi}")
                    nc.scalar.activation(lden[:], den[:, q], LN, bias=zb[:])
                    rden = sbp.tile([1, 256], f32, tag=f"rd{qi}")
                    nc.scalar.activation(
                        rden[:], lden[:], EXP, bias=zb[:], scale=-1.0
                    )
                    bc_sb = sbp.tile([C, 256], f32, tag=f"bc{qi}")
                    nc.gpsimd.partition_broadcast(bc_sb[:], rden[:])
                    fin = sbp.tile([C, 256], f32, tag=f"fin{qi}")
                    nc.vector.tensor_mul(fin[:], out_ps[0:C, q], bc_sb[:])
                    dmas[qi].dma_start(out_e[:, q], fin[:])

    nc.finalize()
    return nc


_NC = None


def _get_nc():
    global _NC
    if _NC is None:
        _NC = _build_nc()
    return _NC


def _in_maps(x, Wq, Wk, Wv):
    x = np.asarray(x, np.float32)
    Wq = np.asarray(Wq, np.float32)
    Wk = np.asarray(Wk, np.float32)
    Wv = np.asarray(Wv, np.float32)
    Ah = _lin_interp_mat(H, 2 * H)
    awT = np.ascontiguousarray(_lin_interp_mat(W, 2 * W).T)  # [32, 64]
    wpack = np.zeros((C, 136), np.float32)
    wpack[:, 0:D] = Wk.T
    wpack[:, D : D + C] = Wv.T
    wpack[0:W, D + C : D + C + 64] = awT
    # G_s[(c rloc), (i, d)] = Wq[d, c] * Ah_s[i, r0+rloc], c-major flat
    # row index (c*RWIN + rloc) split into NCHUNK chunks of 128
    gms = []
    for s in range(4):
        r0 = R_START[s]
        Ah_s = Ah[s * 16 : (s + 1) * 16, r0 : r0 + RWIN]  # [16, RWIN]
        G = np.einsum("dc,ir->crid", Wq, Ah_s).reshape(C * RWIN, 16 * D)
        gms.append(
            np.ascontiguousarray(
                G.reshape(NCHUNK, 128, 128)
                .transpose(1, 0, 2)
                .reshape(128, NCHUNK * 128)
            )
        )
    maps = []
    for i in range(N_CORES):
        b, s = divmod(i, 4)
        r0 = R_START[s]
        xb = np.ascontiguousarray(x[b].reshape(C, H * W))
        xsub = x[b][:, r0 : r0 + RWIN, :]  # [C, RWIN, W]
        xchr = np.ascontiguousarray(
            xsub.reshape(NCHUNK, 128, W).transpose(1, 0, 2).reshape(128, -1)
        )
        maps.append({"xb": xb, "xchr": xchr, "gmat": gms[s], "wpack": wpack})
    return maps


def _run(x, Wq, Wk, Wv, trace=False):
    from concourse.bass_utils import run_bass_kernel_spmd

    nc = _get_nc()
    res = run_bass_kernel_spmd(
        nc, _in_maps(x, Wq, Wk, Wv), core_ids=list(range(N_CORES)), trace=trace
    )
    out = np.empty((B, C, 4 * H * W), np.float32)
    for i in range(N_CORES):
        b, s = divmod(i, 4)
        out[b, :, s * NQ : (s + 1) * NQ] = res.results[i]["out"]
    return out.reshape(B, C, 2 * W, 2 * H), res


def kernel(x, Wq, Wk, Wv):
    out, _ = _run(x, Wq, Wk, Wv)
    return out


# revision 43
# speedup vs baseline: 1.1769x; 1.0021x over previous
"""Trainium2 Bass kernel for nn_AttentionConv (sparse checkerboard attention).

Math (per batch image, C=64, H=W=32, N=4096 upsampled tokens):
  q,k,v = 1x1 convs; q is bilinearly 2x-upsampled, k/v zero-upsampled
  (values only at (even,even) positions).  A checkerboard mask of -1e8 is
  added to k itself, so the 3072 masked key columns are all identically
  (-1e8,...,-1e8): their score for query n is -1e8*S(n) with
  S(n)=sum_d q_up[n,d], and their v is 0.  Hence
     out[c,n] = sum_{m' in 1024 unmasked} v[c,m'] exp(s[n,m']) / D(n)
     D(n)     = 3072*exp(-1e8*S(n)) + sum_{m'} exp(s[n,m'])
  with s[n,m'] = q_up[n,:].k[:,m'].

All exps carry a constant bias of -20 (out = num'/den' is invariant):
den' = den*e^-20 then spans [2e-6, 7e7], inside the domain where the ACT
ln table is accurate (it breaks past ~2^+-64), so 1/den can run on the
ACT engine as exp(-ln(den)) -- the DVE reciprocal is ~8 cycles/element
on a single lane and the ACT Reciprocal function is blocked in bass.

Device pipeline per core (matmul operands f32r; the big v^T.p
accumulation runs in bf16; PSUM accumulates in f32):
  k    = Wk x                                            [8, 1024]
  vT   = x^T Wv^T per 128-token chunk -> vTa (bf16, +ones denom row)
  t2   = kron(Wq, Ah_slice) contraction over the <=10 source rows the
         16-row slice touches (5 chunks of 128)          [32, 128]
  qfT  = col-interp of t2 via awT                        [8, 1024]
  dex  = exp(-1e8*max(S,-4.65e-7) - 20): masked-key denominator term;
         the clamp keeps the S<0 saturation finite (~1e15) because the
         exp table emits NaN for huge positive args
  loop over 8 key tiles: sT = k_t^T qfT -> exp -> bf16 pT ->
         out_ps[65,1024] += [v_t;1]^T pT
  rden = exp(-ln(den)), broadcast on GPSIMD, multiply on DVE, DMA out.

Sharding: 8 cores = 2 batches x 4 query-slices of 1024 tokens
(16 upsampled rows each).  No collectives; each core writes a disjoint
[64, 1024] output slice.
"""
import sys

import numpy as np

if "/opt/trn_rl_repo" not in sys.path:
    sys.path.insert(0, "/opt/trn_rl_repo")

B, C, H, W = 2, 64, 32, 32
D = 8          # q/k head dim
NQ = 1024      # query tokens per core (16 upsampled rows x 64 cols)
NK = 1024      # unmasked keys per image (= H*W)
N_CORES = 8
RWIN = 10      # source rows touched by one 16-row upsampled slice
NCHUNK = (C * RWIN) // 128  # = 5 kron contraction chunks
R_START = (0, 7, 15, 22)    # first source row per slice


def _lin_interp_mat(n_in, n_out):
    # float32 replica of reference's bilinear (align_corners=True) matrix
    pos = np.arange(n_out, dtype=np.float32) * np.float32(
        (n_in - 1) / (n_out - 1)
    )
    i0 = np.clip(np.floor(pos), 0, n_in - 2).astype(np.int32)
    w = (pos - i0.astype(np.float32)).astype(np.float32)
    A = np.zeros((n_out, n_in), np.float32)
    r = np.arange(n_out)
    np.add.at(A, (r, i0), 1.0 - w)
    np.add.at(A, (r, i0 + 1), w)
    return A


def _patched_act_tables(orig):
    """Pin Exp/Ln/Copy to the one table set that holds all of them
    (natural_log_exp_and_others).  The greedy table-load pass otherwise
    ping-pongs between an exp-only and an ln-only set in the epilogue,
    costing 4 extra 1.3us ACT_TABLE_LOADs on the critical path."""
    import functools

    @functools.cache
    def patched(arch):
        tabs = dict(orig(arch))
        combined = None
        for name, s in tabs.items():
            names = {f.name for f in s}
            if {"Exp", "Ln", "Copy"} <= names:
                combined = name
                break
        if combined is None:
            return tabs
        keep = tabs[combined]
        return {
            name: (s if name == combined else s - keep)
            for name, s in tabs.items()
        }

    return patched


def _build_nc():
    import concourse.bacc as bacc
    import concourse.mybir as mybir
    from concourse import tile

    f32 = mybir.dt.float32
    f32r = mybir.dt.float32r
    bf16 = mybir.dt.bfloat16
    EXP = mybir.ActivationFunctionType.Exp
    LN = mybir.ActivationFunctionType.Ln

    orig_tables = bacc.get_activation_tables
    bacc.get_activation_tables = _patched_act_tables(orig_tables)
    try:
        return _build_nc_inner(bacc, mybir, tile, f32, f32r, bf16, EXP, LN)
    finally:
        bacc.get_activation_tables = orig_tables


def _build_nc_inner(bacc, mybir, tile, f32, f32r, bf16, EXP, LN):
    f16 = mybir.dt.float16
    nc = bacc.Bacc(None, target_bir_lowering=False)

    xb_e = nc.declare_dram_parameter("xb", [C, H * W], f32r, isOutput=False)
    xchr_e = nc.declare_dram_parameter(
        "xchr", [128, NCHUNK * W], f32r, isOutput=False
    )
    gm_e = nc.declare_dram_parameter(
        "gmat", [128, NCHUNK * 128], f32r, isOutput=False
    )
    wp_e = nc.declare_dram_parameter("wpack", [C, 136], f32r, isOutput=False)
    out_e = nc.declare_dram_parameter("out", [C, NQ], f32, isOutput=True)

    hA = slice(0, 512)
    hB = slice(512, 1024)

    with tile.TileContext(nc) as tc:
        with (
            nc.allow_low_precision(
                reason="f32r/bf16 matmul operands; PSUM accumulates in f32"
            ),
            tc.tile_pool(name="const", bufs=1) as cst,
            tc.tile_pool(name="sb", bufs=1) as sbp,
            tc.tile_pool(name="pexp", bufs=3) as pexp,
        ):
            # ---- input DMAs.  Each queue pays ~2.4us startup latency on
            # its first transfer then streams at ~200GB/s, so xb halves
            # lead the scalar/gpsimd queues and everything lands ~11us.
            xb = cst.tile([C, H * W], f32r)
            nc.scalar.dma_start(xb[:, hA], xb_e[:, hA])
            nc.gpsimd.dma_start(xb[:, hB], xb_e[:, hB])
            wpack = cst.tile([C, 136], f32r)
            nc.sync.dma_start(wpack[:], wp_e[:])
            xchr = cst.tile([128, NCHUNK * W], f32r)
            nc.sync.dma_start(xchr[:], xchr_e[:])
            gm = cst.tile([128, NCHUNK * 128], f32r)
            nc.scalar.dma_start(gm[:, 0 : 2 * 128], gm_e[:, 0 : 2 * 128])
            nc.gpsimd.dma_start(gm[:, 2 * 128 : 4 * 128], gm_e[:, 2 * 128 : 4 * 128])
            nc.sync.dma_start(gm[:, 4 * 128 : 5 * 128], gm_e[:, 4 * 128 : 5 * 128])
            wk = wpack[:, 0:D]
            wv = wpack[:, D : D + C]
            awT = wpack[0:W, D + C : D + C + 64]

            # ---- small constants (memset f32, cast to f32r) ----
            cf = cst.tile([D, 1], f32)
            nc.vector.memset(cf[:], 1.0)
            ones8 = cst.tile([D, 1], f32r)
            nc.vector.tensor_copy(ones8[:], cf[:])
            e65f = cst.tile([1, C + 1], f32)
            nc.vector.memset(e65f[:], 0.0)
            nc.vector.memset(e65f[:, C : C + 1], 3072.0)
            e65 = cst.tile([1, C + 1], f32r)
            nc.vector.tensor_copy(e65[:], e65f[:])
            zb = cst.tile([1, 1], f32)
            nc.vector.memset(zb[:], 0.0)
            # constant bias -20 on every score exp; see module docstring
            mB = cst.tile([128, 1], f32)
            nc.vector.memset(mB[:], -20.0)

            # ---- persistent SBUF tiles ----
            k_sb = sbp.tile([D, NK], f16)
            qfT = sbp.tile([D, NQ], f32r)   # f32r: S-row summation only
            qfT16 = sbp.tile([D, NQ], f16)  # fp16: score matmul rhs
            dex = sbp.tile([1, NQ], f32r)
            s_cl = sbp.tile([1, NQ], f32)
            vTa = sbp.tile([128, 8 * (C + 1)], bf16)
            nc.vector.memset(
                vTa[:].rearrange("p (t c) -> p t c", c=C + 1)[:, :, C], 1.0
            )

            with tc.tile_pool(name="ps_o", bufs=1, space="PSUM") as pso:
                out_ps = pso.tile([C + 1, NQ], f32)

                # ---- setup matmuls (all f32r) ----
                with tc.tile_pool(name="ps_s", bufs=1, space="PSUM") as pss:
                    # k projection first: only needs xb + wpack.  One
                    # PSUM bank, halves serialized on the copy (off the
                    # critical path); v tiles (double-buffered) fill PE
                    # while the kron operands stream in.
                    def vt_tile(u):
                        vt_ps = pss.tile(
                            [128, C], f32, tag="vtA" if u % 2 == 0 else "vtB"
                        )
                        nc.tensor.matmul(
                            vt_ps[:],
                            xb[:, u * 128 : (u + 1) * 128],
                            wv,
                            start=True,
                            stop=True,
                        )
                        if u % 2 == 0:
                            nc.vector.tensor_copy(
                                vTa[:, u * (C + 1) : u * (C + 1) + C],
                                vt_ps[:],
                            )
                        else:
                            nc.scalar.copy(
                                vTa[:, u * (C + 1) : u * (C + 1) + C],
                                vt_ps[:],
                            )

                    k_psA = pss.tile([D, 512], f32, tag="kps")
                    nc.tensor.matmul(
                        k_psA[:], wk, xb[:, hA], start=True, stop=True
                    )
                    nc.vector.tensor_copy(k_sb[:, hA], k_psA[:])
                    for u in range(3):
                        vt_tile(u)
                    k_psB = pss.tile([D, 512], f32, tag="kps")
                    nc.tensor.matmul(
                        k_psB[:], wk, xb[:, hB], start=True, stop=True
                    )
                    nc.scalar.copy(k_sb[:, hB], k_psB[:])

                    # fused q-proj + row-interp (kron), then col-interp
                    t2_ps = pss.tile([W, 128], f32, tag="t2")
                    for k in range(NCHUNK):
                        nc.tensor.matmul(
                            t2_ps[:],
                            xchr[:, k * W : (k + 1) * W],
                            gm[:, k * 128 : (k + 1) * 128],
                            start=(k == 0),
                            stop=(k == NCHUNK - 1),
                            skip_group_check=True,
                        )
                    t2_sb = sbp.tile([W, 128], f32r)
                    nc.scalar.copy(t2_sb[:], t2_ps[:])
                    for u in range(3, 8):
                        vt_tile(u)

                    # col-interp in two independent banks so each half's
                    # PSUM->SBUF copy starts as soon as its 8 matmuls end
                    qf_psA = pss.tile([D, 512], f32, tag="qfA")
                    qf_psB = pss.tile([D, 512], f32, tag="qfB")
                    t2_v = t2_sb[:].rearrange("c (i d) -> c i d", i=16)
                    for i in range(16):
                        dst = qf_psA if i < 8 else qf_psB
                        nc.tensor.matmul(
                            dst[:, (i % 8) * 64 : (i % 8) * 64 + 64],
                            t2_v[:, i, :],
                            awT,
                            start=True,
                            stop=True,
                        )
                    nc.vector.tensor_copy(qfT16[:, hA], qf_psA[:])
                    nc.scalar.copy(qfT16[:, hB], qf_psB[:])
                    nc.vector.tensor_copy(qfT[:, hA], qf_psA[:])
                    nc.scalar.copy(qfT[:, hB], qf_psB[:])


                # ---- main loop: scores^T, exp, accumulate ----
                with tc.tile_pool(
                    name="ps_sc", bufs=2, space="PSUM"
                ) as pssc:
                    for t in range(8):
                        sT = pssc.tile([128, NQ], f32)
                        kT_t = k_sb[:, t * 128 : (t + 1) * 128]
                        for h in (hA, hB):
                            nc.tensor.matmul(
                                sT[:, h], kT_t, qfT16[:, h], start=True,
                                stop=True,
                            )
                        if t == 0:
                            # S = colsum(qfT) + clamp, in the exp0 shadow
                            # (PE idles waiting for pT otherwise).  Summing
                            # the same f32r qfT the scores use keeps the
                            # sign of borderline-|S| queries aligned with
                            # the reference's masked-row selection.
                            for h in (hA, hB):
                                s_ps = pssc.tile([1, 512], f32, tag="srow")
                                nc.tensor.matmul(
                                    s_ps[:], ones8[:], qfT[:, h],
                                    start=True, stop=True,
                                )
                                nc.vector.tensor_scalar(
                                    s_cl[:, h], s_ps[:], -4.65e-7, None,
                                    mybir.AluOpType.max,
                                )
                        pT = pexp.tile([128, NQ], bf16, tag="pT")
                        if t < 7:
                            nc.scalar.activation(
                                pT[:], sT[:], EXP, bias=mB[:]
                            )
                        else:
                            # halved so out7A (stop) lands earlier and the
                            # A-half epilogue overlaps the B-half tail
                            nc.scalar.activation(
                                pT[:, hA], sT[:, hA], EXP, bias=mB[:]
                            )
                            nc.scalar.activation(
                                pT[:, hB], sT[:, hB], EXP, bias=mB[:]
                            )
                        if t == 5:
                            # masked-key denominator term, off the
                            # critical path
                            nc.scalar.activation(
                                dex[:], s_cl[:], EXP, bias=mB[0:1, :],
                                scale=-1.0e8,
                            )
                        for h in (hA, hB):
                            nc.tensor.matmul(
                                out_ps[:, h],
                                vTa[:, t * (C + 1) : (t + 1) * (C + 1)],
                                pT[:, h],
                                start=(t == 0),
                                stop=(t == 7),
                                skip_group_check=True,
                            )
                        if t == 5:
                            for h in (hA, hB):
                                nc.tensor.matmul(
                                    out_ps[:, h],
                                    e65[:],
                                    dex[:, h],
                                    start=False,
                                    stop=False,
                                    skip_group_check=True,
                                )

                # ---- epilogue: rden = exp(-ln(den)) on ACT, broadcast
                # on GPSIMD (SBUF->SBUF, so the DVE multiply's only PSUM
                # operand is num), multiply on DVE, DMA out.  Quartered
                # and fully per-quarter tiles so the chain pipelines. ----
                den = out_ps[C : C + 1, :]
                dmas = (nc.sync, nc.sync, nc.sync, nc.sync)
                for qi in range(4):
                    q = slice(qi * 256, (qi + 1) * 256)
                    lden = sbp.tile([1, 256], f32, tag=f"ld{qi}")
                    nc.scalar.activation(lden[:], den[:, q], LN, bias=zb[:])
                    rden = sbp.tile([1, 256], f32, tag=f"rd{qi}")
                    nc.scalar.activation(
                        rden[:], lden[:], EXP, bias=zb[:], scale=-1.0
                    )
                    bc_sb = sbp.tile([C, 256], f32, tag=f"bc{qi}")
                    nc.gpsimd.partition_broadcast(bc_sb[:], rden[:])
                    fin = sbp.tile([C, 256], f32, tag=f"fin{qi}")
                    nc.vector.tensor_mul(fin[:], out_ps[0:C, q], bc_sb[:])
                    dmas[qi].dma_start(out_e[:, q], fin[:])

    nc.finalize()
    return nc


_NC = None


def _get_nc():
    global _NC
    if _NC is None:
        _NC = _build_nc()
    return _NC


def _in_maps(x, Wq, Wk, Wv):
    x = np.asarray(x, np.float32)
    Wq = np.asarray(Wq, np.float32)
    Wk = np.asarray(Wk, np.float32)
    Wv = np.asarray(Wv, np.float32)
    Ah = _lin_interp_mat(H, 2 * H)
    awT = np.ascontiguousarray(_lin_interp_mat(W, 2 * W).T)  # [32, 64]
    wpack = np.zeros((C, 136), np.float32)
    wpack[:, 0:D] = Wk.T
    wpack[:, D : D + C] = Wv.T
    wpack[0:W, D + C : D + C + 64] = awT
    # G_s[(c rloc), (i, d)] = Wq[d, c] * Ah_s[i, r0+rloc], c-major flat
    # row index (c*RWIN + rloc) split into NCHUNK chunks of 128
    gms = []
    for s in range(4):
        r0 = R_START[s]
        Ah_s = Ah[s * 16 : (s + 1) * 16, r0 : r0 + RWIN]  # [16, RWIN]
        G = np.einsum("dc,ir->crid", Wq, Ah_s).reshape(C * RWIN, 16 * D)
        gms.append(
            np.ascontiguousarray(
                G.reshape(NCHUNK, 128, 128)
                .transpose(1, 0, 2)
                .reshape(128, NCHUNK * 128)
            )
        )
    maps = []
    for i in range(N_CORES):
        b, s = divmod(i, 4)
        r0 = R_START[s]
        xb = np.ascontiguousarray(x[b].reshape(C, H * W))
        xsub = x[b][:, r0 : r0 + RWIN, :]  # [C, RWIN, W]
        xchr = np.ascontiguousarray(
            xsub.reshape(NCHUNK, 128, W).transpose(1, 0, 2).reshape(128, -1)
        )
        maps.append({"xb": xb, "xchr": xchr, "gmat": gms[s], "wpack": wpack})
    return maps


def _run(x, Wq, Wk, Wv, trace=False):
    from concourse.bass_utils import run_bass_kernel_spmd

    nc = _get_nc()
    res = run_bass_kernel_spmd(
        nc, _in_maps(x, Wq, Wk, Wv), core_ids=list(range(N_CORES)), trace=trace
    )
    out = np.empty((B, C, 4 * H * W), np.float32)
    for i in range(N_CORES):
        b, s = divmod(i, 4)
        out[b, :, s * NQ : (s + 1) * NQ] = res.results[i]["out"]
    return out.reshape(B, C, 2 * W, 2 * H), res


def kernel(x, Wq, Wk, Wv):
    out, _ = _run(x, Wq, Wk, Wv)
    return out


# revision 46
# speedup vs baseline: 1.2868x; 1.0934x over previous
"""Trainium2 Bass kernel for nn_AttentionConv (sparse checkerboard attention).

Math (per batch image, C=64, H=W=32, N=4096 upsampled tokens):
  q,k,v = 1x1 convs; q is bilinearly 2x-upsampled, k/v zero-upsampled
  (values only at (even,even) positions).  A checkerboard mask of -1e8 is
  added to k itself, so the 3072 masked key columns are all identically
  (-1e8,...,-1e8): their score for query n is -1e8*S(n) with
  S(n)=sum_d q_up[n,d], and their v is 0.  Hence
     out[c,n] = sum_{m' in 1024 unmasked} v[c,m'] exp(s[n,m']) / D(n)
     D(n)     = 3072*exp(-1e8*S(n)) + sum_{m'} exp(s[n,m'])
  with s[n,m'] = q_up[n,:].k[:,m'].

All exps carry a constant bias of -20 (out = num'/den' is invariant):
den' = den*e^-20 then spans [2e-6, 7e7], inside the domain where the ACT
ln table is accurate (it breaks past ~2^+-64), so 1/den can run on the
ACT engine as exp(-ln(den)) -- the DVE reciprocal is ~8 cycles/element
on a single lane and the ACT Reciprocal function is blocked in bass.

Device pipeline per core (matmul operands f32r; the big v^T.p
accumulation runs in bf16; PSUM accumulates in f32):
  k    = Wk x                                            [8, 1024]
  vT   = x^T Wv^T per 128-token chunk -> vTa (bf16, +ones denom row)
  t2   = kron(Wq, Ah_slice) contraction over the <=10 source rows the
         16-row slice touches (5 chunks of 128)          [32, 128]
  qfT  = col-interp of t2 via awT                        [8, 1024]
  dex  = exp(-1e8*max(S,-4.65e-7) - 20): masked-key denominator term;
         the clamp keeps the S<0 saturation finite (~1e15) because the
         exp table emits NaN for huge positive args
  loop over 8 key tiles: sT = k_t^T qfT -> exp -> bf16 pT ->
         out_ps[65,1024] += [v_t;1]^T pT
  rden = exp(-ln(den)), broadcast on GPSIMD, multiply on DVE, DMA out.

Sharding: 8 cores = 2 batches x 4 query-slices of 1024 tokens
(16 upsampled rows each).  No collectives; each core writes a disjoint
[64, 1024] output slice.
"""
import sys

import numpy as np

if "/opt/trn_rl_repo" not in sys.path:
    sys.path.insert(0, "/opt/trn_rl_repo")

B, C, H, W = 2, 64, 32, 32
D = 8          # q/k head dim
NQ = 1024      # query tokens per core (16 upsampled rows x 64 cols)
NK = 1024      # unmasked keys per image (= H*W)
N_CORES = 8
RWIN = 10      # source rows touched by one 16-row upsampled slice
NCHUNK = (C * RWIN) // 128  # = 5 kron contraction chunks
R_START = (0, 7, 15, 22)    # first source row per slice


def _lin_interp_mat(n_in, n_out):
    # float32 replica of reference's bilinear (align_corners=True) matrix
    pos = np.arange(n_out, dtype=np.float32) * np.float32(
        (n_in - 1) / (n_out - 1)
    )
    i0 = np.clip(np.floor(pos), 0, n_in - 2).astype(np.int32)
    w = (pos - i0.astype(np.float32)).astype(np.float32)
    A = np.zeros((n_out, n_in), np.float32)
    r = np.arange(n_out)
    np.add.at(A, (r, i0), 1.0 - w)
    np.add.at(A, (r, i0 + 1), w)
    return A


def _patched_act_tables(orig):
    """Pin Exp/Ln/Copy to the one table set that holds all of them
    (natural_log_exp_and_others).  The greedy table-load pass otherwise
    ping-pongs between an exp-only and an ln-only set in the epilogue,
    costing 4 extra 1.3us ACT_TABLE_LOADs on the critical path."""
    import functools

    @functools.cache
    def patched(arch):
        tabs = dict(orig(arch))
        combined = None
        for name, s in tabs.items():
            names = {f.name for f in s}
            if {"Exp", "Ln", "Copy"} <= names:
                combined = name
                break
        if combined is None:
            return tabs
        keep = tabs[combined]
        return {
            name: (s if name == combined else s - keep)
            for name, s in tabs.items()
        }

    return patched


def _build_nc():
    import concourse.bacc as bacc
    import concourse.mybir as mybir
    from concourse import tile

    f32 = mybir.dt.float32
    f32r = mybir.dt.float32r
    bf16 = mybir.dt.bfloat16
    EXP = mybir.ActivationFunctionType.Exp
    LN = mybir.ActivationFunctionType.Ln

    orig_tables = bacc.get_activation_tables
    bacc.get_activation_tables = _patched_act_tables(orig_tables)
    try:
        return _build_nc_inner(bacc, mybir, tile, f32, f32r, bf16, EXP, LN)
    finally:
        bacc.get_activation_tables = orig_tables


def _build_nc_inner(bacc, mybir, tile, f32, f32r, bf16, EXP, LN):
    f16 = mybir.dt.float16
    nc = bacc.Bacc(None, target_bir_lowering=False)

    xb_e = nc.declare_dram_parameter("xb", [C, H * W], f32r, isOutput=False)
    xchr_e = nc.declare_dram_parameter(
        "xchr", [128, NCHUNK * W], f32r, isOutput=False
    )
    gm_e = nc.declare_dram_parameter(
        "gmat", [128, NCHUNK * 128], f32r, isOutput=False
    )
    wp_e = nc.declare_dram_parameter("wpack", [C, 136], f32r, isOutput=False)
    xb16_e = nc.declare_dram_parameter("xb16", [C, H * W], f16, isOutput=False)
    wv16_e = nc.declare_dram_parameter("wv16", [C, C], f16, isOutput=False)
    out_e = nc.declare_dram_parameter("out", [C, NQ], f32, isOutput=True)

    hA = slice(0, 512)
    hB = slice(512, 1024)

    with tile.TileContext(nc) as tc:
        with (
            nc.allow_low_precision(
                reason="f32r/bf16 matmul operands; PSUM accumulates in f32"
            ),
            tc.tile_pool(name="const", bufs=1) as cst,
            tc.tile_pool(name="sb", bufs=1) as sbp,
            tc.tile_pool(name="pexp", bufs=3) as pexp,
        ):
            # ---- input DMAs.  Each queue pays ~2.4us startup latency on
            # its first transfer then streams at ~200GB/s, so xb halves
            # lead the scalar/gpsimd queues and everything lands ~11us.
            xb = cst.tile([C, H * W], f32r)
            nc.scalar.dma_start(xb[:, hA], xb_e[:, hA])
            nc.gpsimd.dma_start(xb[:, hB], xb_e[:, hB])
            wpack = cst.tile([C, 136], f32r)
            nc.sync.dma_start(wpack[:], wp_e[:])
            xchr = cst.tile([128, NCHUNK * W], f32r)
            nc.sync.dma_start(xchr[:], xchr_e[:])
            wv16 = cst.tile([C, C], f16)
            nc.sync.dma_start(wv16[:], wv16_e[:])
            xb16 = cst.tile([C, H * W], f16)
            nc.sync.dma_start(xb16[:], xb16_e[:])
            gm = cst.tile([128, NCHUNK * 128], f32r)
            nc.scalar.dma_start(gm[:, 0 : 2 * 128], gm_e[:, 0 : 2 * 128])
            nc.gpsimd.dma_start(gm[:, 2 * 128 : 4 * 128], gm_e[:, 2 * 128 : 4 * 128])
            nc.sync.dma_start(gm[:, 4 * 128 : 5 * 128], gm_e[:, 4 * 128 : 5 * 128])
            wk = wpack[:, 0:D]
            wv = wpack[:, D : D + C]
            awT = wpack[0:W, D + C : D + C + 64]

            # ---- small constants (memset f32, cast to f32r) ----
            cf = cst.tile([D, 1], f32)
            nc.vector.memset(cf[:], 1.0)
            ones8 = cst.tile([D, 1], f32r)
            nc.vector.tensor_copy(ones8[:], cf[:])
            e65f = cst.tile([1, C + 1], f32)
            nc.vector.memset(e65f[:], 0.0)
            nc.vector.memset(e65f[:, C : C + 1], 3072.0)
            e65 = cst.tile([1, C + 1], f32r)
            nc.vector.tensor_copy(e65[:], e65f[:])
            zb = cst.tile([1, 1], f32)
            nc.vector.memset(zb[:], 0.0)
            # constant bias -20 on every score exp; see module docstring
            mB = cst.tile([128, 1], f32)
            nc.vector.memset(mB[:], -20.0)

            # ---- persistent SBUF tiles ----
            k_sb = sbp.tile([D, NK], f16)
            qfT = sbp.tile([D, NQ], f32r)   # f32r: S-row summation only
            qfT16 = sbp.tile([D, NQ], f16)  # fp16: score matmul rhs
            dex = sbp.tile([1, NQ], f32r)
            s_cl = sbp.tile([1, NQ], f32)
            vTa = sbp.tile([128, 8 * (C + 1)], bf16)
            nc.vector.memset(
                vTa[:].rearrange("p (t c) -> p t c", c=C + 1)[:, :, C], 1.0
            )

            with tc.tile_pool(name="ps_o", bufs=1, space="PSUM") as pso:
                out_ps = pso.tile([C + 1, NQ], f32)

                # ---- setup matmuls (all f32r) ----
                with tc.tile_pool(name="ps_s", bufs=1, space="PSUM") as pss:
                    # k projection first: only needs xb + wpack.  One
                    # PSUM bank, halves serialized on the copy (off the
                    # critical path); v tiles (double-buffered) fill PE
                    # while the kron operands stream in.
                    def vt_tile(u):
                        vt_ps = pss.tile(
                            [128, C], f32, tag="vtA" if u % 2 == 0 else "vtB"
                        )
                        nc.tensor.matmul(
                            vt_ps[:],
                            xb16[:, u * 128 : (u + 1) * 128],
                            wv16[:],
                            start=True,
                            stop=True,
                        )
                        if u % 2 == 0:
                            nc.vector.tensor_copy(
                                vTa[:, u * (C + 1) : u * (C + 1) + C],
                                vt_ps[:],
                            )
                        else:
                            nc.scalar.copy(
                                vTa[:, u * (C + 1) : u * (C + 1) + C],
                                vt_ps[:],
                            )

                    k_psA = pss.tile([D, 512], f32, tag="kps")
                    nc.tensor.matmul(
                        k_psA[:], wk, xb[:, hA], start=True, stop=True
                    )
                    nc.vector.tensor_copy(k_sb[:, hA], k_psA[:])
                    for u in range(3):
                        vt_tile(u)
                    k_psB = pss.tile([D, 512], f32, tag="kps")
                    nc.tensor.matmul(
                        k_psB[:], wk, xb[:, hB], start=True, stop=True
                    )
                    nc.scalar.copy(k_sb[:, hB], k_psB[:])

                    # fused q-proj + row-interp (kron), then col-interp
                    t2_ps = pss.tile([W, 128], f32, tag="t2")
                    for k in range(NCHUNK):
                        nc.tensor.matmul(
                            t2_ps[:],
                            xchr[:, k * W : (k + 1) * W],
                            gm[:, k * 128 : (k + 1) * 128],
                            start=(k == 0),
                            stop=(k == NCHUNK - 1),
                            skip_group_check=True,
                        )
                    t2_sb = sbp.tile([W, 128], f32r)
                    nc.scalar.copy(t2_sb[:], t2_ps[:])
                    for u in range(3, 8):
                        vt_tile(u)

                    # col-interp in two independent banks so each half's
                    # PSUM->SBUF copy starts as soon as its 8 matmuls end
                    qf_psA = pss.tile([D, 512], f32, tag="qfA")
                    qf_psB = pss.tile([D, 512], f32, tag="qfB")
                    t2_v = t2_sb[:].rearrange("c (i d) -> c i d", i=16)
                    for i in range(16):
                        dst = qf_psA if i < 8 else qf_psB
                        nc.tensor.matmul(
                            dst[:, (i % 8) * 64 : (i % 8) * 64 + 64],
                            t2_v[:, i, :],
                            awT,
                            start=True,
                            stop=True,
                        )
                    nc.vector.tensor_copy(qfT16[:, hA], qf_psA[:])
                    nc.scalar.copy(qfT16[:, hB], qf_psB[:])
                    nc.vector.tensor_copy(qfT[:, hA], qf_psA[:])
                    nc.scalar.copy(qfT[:, hB], qf_psB[:])


                # ---- main loop: scores^T, exp, accumulate ----
                with tc.tile_pool(
                    name="ps_sc", bufs=2, space="PSUM"
                ) as pssc:
                    for t in range(8):
                        sT = pssc.tile([128, NQ], f32)
                        kT_t = k_sb[:, t * 128 : (t + 1) * 128]
                        for h in (hA, hB):
                            nc.tensor.matmul(
                                sT[:, h], kT_t, qfT16[:, h], start=True,
                                stop=True,
                            )
                        if t == 0:
                            # S = colsum(qfT) + clamp, in the exp0 shadow
                            # (PE idles waiting for pT otherwise).  Summing
                            # the same f32r qfT the scores use keeps the
                            # sign of borderline-|S| queries aligned with
                            # the reference's masked-row selection.
                            for h in (hA, hB):
                                s_ps = pssc.tile([1, 512], f32, tag="srow")
                                nc.tensor.matmul(
                                    s_ps[:], ones8[:], qfT[:, h],
                                    start=True, stop=True,
                                )
                                nc.vector.tensor_scalar(
                                    s_cl[:, h], s_ps[:], -4.65e-7, None,
                                    mybir.AluOpType.max,
                                )
                        pT = pexp.tile([128, NQ], bf16, tag="pT")
                        if t < 7:
                            nc.scalar.activation(
                                pT[:], sT[:], EXP, bias=mB[:]
                            )
                        else:
                            # halved so out7A (stop) lands earlier and the
                            # A-half epilogue overlaps the B-half tail
                            nc.scalar.activation(
                                pT[:, hA], sT[:, hA], EXP, bias=mB[:]
                            )
                            nc.scalar.activation(
                                pT[:, hB], sT[:, hB], EXP, bias=mB[:]
                            )
                        if t == 5:
                            # masked-key denominator term, off the
                            # critical path
                            nc.scalar.activation(
                                dex[:], s_cl[:], EXP, bias=mB[0:1, :],
                                scale=-1.0e8,
                            )
                        for h in (hA, hB):
                            nc.tensor.matmul(
                                out_ps[:, h],
                                vTa[:, t * (C + 1) : (t + 1) * (C + 1)],
                                pT[:, h],
                                start=(t == 0),
                                stop=(t == 7),
                                skip_group_check=True,
                            )
                        if t == 5:
                            for h in (hA, hB):
                                nc.tensor.matmul(
                                    out_ps[:, h],
                                    e65[:],
                                    dex[:, h],
                                    start=False,
                                    stop=False,
                                    skip_group_check=True,
                                )

                # ---- epilogue: rden = exp(-ln(den)) on ACT in halves
                # (ACT per-op overhead is ~270ns), broadcast on GPSIMD
                # and multiply on DVE in quarters for pipelining.  Reads
                # of out_ps serialize pairwise across engines, so the ln
                # halves and the mult quarters are emitted grouped. ----
                den = out_ps[C : C + 1, :]
                ldenA = sbp.tile([1, 512], f32, tag="ldA")
                rdenA = sbp.tile([1, 512], f32, tag="rdA")
                ldenB = sbp.tile([1, 512], f32, tag="ldB")
                rdenB = sbp.tile([1, 512], f32, tag="rdB")
                nc.scalar.activation(ldenA[:], den[:, hA], LN, bias=zb[:])
                nc.scalar.activation(
                    rdenA[:], ldenA[:], EXP, bias=zb[:], scale=-1.0
                )
                nc.scalar.activation(ldenB[:], den[:, hB], LN, bias=zb[:])
                nc.scalar.activation(
                    rdenB[:], ldenB[:], EXP, bias=zb[:], scale=-1.0
                )
                rdens = (rdenA, rdenA, rdenB, rdenB)
                bcs, fins = [], []
                for qi in range(4):
                    bc_sb = sbp.tile([C, 256], f32, tag=f"bc{qi}")
                    nc.gpsimd.partition_broadcast(
                        bc_sb[:], rdens[qi][:, (qi % 2) * 256 : (qi % 2) * 256 + 256]
                    )
                    bcs.append(bc_sb)
                for qi in range(4):
                    q = slice(qi * 256, (qi + 1) * 256)
                    fin = sbp.tile([C, 256], f32, tag=f"fin{qi}")
                    nc.vector.tensor_mul(fin[:], out_ps[0:C, q], bcs[qi][:])
                    fins.append(fin)
                for qi in range(4):
                    q = slice(qi * 256, (qi + 1) * 256)
                    nc.sync.dma_start(out_e[:, q], fins[qi][:])

    nc.finalize()
    return nc


_NC = None


def _get_nc():
    global _NC
    if _NC is None:
        _NC = _build_nc()
    return _NC


def _in_maps(x, Wq, Wk, Wv):
    x = np.asarray(x, np.float32)
    Wq = np.asarray(Wq, np.float32)
    Wk = np.asarray(Wk, np.float32)
    Wv = np.asarray(Wv, np.float32)
    Ah = _lin_interp_mat(H, 2 * H)
    awT = np.ascontiguousarray(_lin_interp_mat(W, 2 * W).T)  # [32, 64]
    wpack = np.zeros((C, 136), np.float32)
    wpack[:, 0:D] = Wk.T
    wpack[:, D : D + C] = Wv.T
    wpack[0:W, D + C : D + C + 64] = awT
    # G_s[(c rloc), (i, d)] = Wq[d, c] * Ah_s[i, r0+rloc], c-major flat
    # row index (c*RWIN + rloc) split into NCHUNK chunks of 128
    gms = []
    for s in range(4):
        r0 = R_START[s]
        Ah_s = Ah[s * 16 : (s + 1) * 16, r0 : r0 + RWIN]  # [16, RWIN]
        G = np.einsum("dc,ir->crid", Wq, Ah_s).reshape(C * RWIN, 16 * D)
        gms.append(
            np.ascontiguousarray(
                G.reshape(NCHUNK, 128, 128)
                .transpose(1, 0, 2)
                .reshape(128, NCHUNK * 128)
            )
        )
    maps = []
    for i in range(N_CORES):
        b, s = divmod(i, 4)
        r0 = R_START[s]
        xb = np.ascontiguousarray(x[b].reshape(C, H * W))
        xsub = x[b][:, r0 : r0 + RWIN, :]  # [C, RWIN, W]
        xchr = np.ascontiguousarray(
            xsub.reshape(NCHUNK, 128, W).transpose(1, 0, 2).reshape(128, -1)
        )
        maps.append(
            {
                "xb": xb,
                "xb16": xb.astype(np.float16),
                "xchr": xchr,
                "gmat": gms[s],
                "wpack": wpack,
                "wv16": wpack[:, D : D + C].astype(np.float16),
            }
        )
    return maps


def _run(x, Wq, Wk, Wv, trace=False):
    from concourse.bass_utils import run_bass_kernel_spmd

    nc = _get_nc()
    res = run_bass_kernel_spmd(
        nc, _in_maps(x, Wq, Wk, Wv), core_ids=list(range(N_CORES)), trace=trace
    )
    out = np.empty((B, C, 4 * H * W), np.float32)
    for i in range(N_CORES):
        b, s = divmod(i, 4)
        out[b, :, s * NQ : (s + 1) * NQ] = res.results[i]["out"]
    return out.reshape(B, C, 2 * W, 2 * H), res


def kernel(x, Wq, Wk, Wv):
    out, _ = _run(x, Wq, Wk, Wv)
    return out


# revision 47
# speedup vs baseline: 1.3048x; 1.0140x over previous
"""Trainium2 Bass kernel for nn_AttentionConv (sparse checkerboard attention).

Math (per batch image, C=64, H=W=32, N=4096 upsampled tokens):
  q,k,v = 1x1 convs; q is bilinearly 2x-upsampled, k/v zero-upsampled
  (values only at (even,even) positions).  A checkerboard mask of -1e8 is
  added to k itself, so the 3072 masked key columns are all identically
  (-1e8,...,-1e8): their score for query n is -1e8*S(n) with
  S(n)=sum_d q_up[n,d], and their v is 0.  Hence
     out[c,n] = sum_{m' in 1024 unmasked} v[c,m'] exp(s[n,m']) / D(n)
     D(n)     = 3072*exp(-1e8*S(n)) + sum_{m'} exp(s[n,m'])
  with s[n,m'] = q_up[n,:].k[:,m'].

All exps carry a constant bias of -20 (out = num'/den' is invariant):
den' = den*e^-20 then spans [2e-6, 7e7], inside the domain where the ACT
ln table is accurate (it breaks past ~2^+-64), so 1/den can run on the
ACT engine as exp(-ln(den)) -- the DVE reciprocal is ~8 cycles/element
on a single lane and the ACT Reciprocal function is blocked in bass.

Device pipeline per core (matmul operands f32r; the big v^T.p
accumulation runs in bf16; PSUM accumulates in f32):
  k    = Wk x                                            [8, 1024]
  vT   = x^T Wv^T per 128-token chunk -> vTa (bf16, +ones denom row)
  t2   = kron(Wq, Ah_slice) contraction over the <=10 source rows the
         16-row slice touches (5 chunks of 128)          [32, 128]
  qfT  = col-interp of t2 via awT                        [8, 1024]
  dex  = exp(-1e8*max(S,-4.65e-7) - 20): masked-key denominator term;
         the clamp keeps the S<0 saturation finite (~1e15) because the
         exp table emits NaN for huge positive args
  loop over 8 key tiles: sT = k_t^T qfT -> exp -> bf16 pT ->
         out_ps[65,1024] += [v_t;1]^T pT
  rden = exp(-ln(den)), broadcast on GPSIMD, multiply on DVE, DMA out.

Sharding: 8 cores = 2 batches x 4 query-slices of 1024 tokens
(16 upsampled rows each).  No collectives; each core writes a disjoint
[64, 1024] output slice.
"""
import sys

import numpy as np

if "/opt/trn_rl_repo" not in sys.path:
    sys.path.insert(0, "/opt/trn_rl_repo")

B, C, H, W = 2, 64, 32, 32
D = 8          # q/k head dim
NQ = 1024      # query tokens per core (16 upsampled rows x 64 cols)
NK = 1024      # unmasked keys per image (= H*W)
N_CORES = 8
RWIN = 10      # source rows touched by one 16-row upsampled slice
NCHUNK = (C * RWIN) // 128  # = 5 kron contraction chunks
R_START = (0, 7, 15, 22)    # first source row per slice


def _lin_interp_mat(n_in, n_out):
    # float32 replica of reference's bilinear (align_corners=True) matrix
    pos = np.arange(n_out, dtype=np.float32) * np.float32(
        (n_in - 1) / (n_out - 1)
    )
    i0 = np.clip(np.floor(pos), 0, n_in - 2).astype(np.int32)
    w = (pos - i0.astype(np.float32)).astype(np.float32)
    A = np.zeros((n_out, n_in), np.float32)
    r = np.arange(n_out)
    np.add.at(A, (r, i0), 1.0 - w)
    np.add.at(A, (r, i0 + 1), w)
    return A


def _patched_act_tables(orig):
    """Pin Exp/Ln/Copy to the one table set that holds all of them
    (natural_log_exp_and_others).  The greedy table-load pass otherwise
    ping-pongs between an exp-only and an ln-only set in the epilogue,
    costing 4 extra 1.3us ACT_TABLE_LOADs on the critical path."""
    import functools

    @functools.cache
    def patched(arch):
        tabs = dict(orig(arch))
        combined = None
        for name, s in tabs.items():
            names = {f.name for f in s}
            if {"Exp", "Ln", "Copy"} <= names:
                combined = name
                break
        if combined is None:
            return tabs
        keep = tabs[combined]
        return {
            name: (s if name == combined else s - keep)
            for name, s in tabs.items()
        }

    return patched


def _build_nc():
    import concourse.bacc as bacc
    import concourse.mybir as mybir
    from concourse import tile

    f32 = mybir.dt.float32
    f32r = mybir.dt.float32r
    bf16 = mybir.dt.bfloat16
    EXP = mybir.ActivationFunctionType.Exp
    LN = mybir.ActivationFunctionType.Ln

    orig_tables = bacc.get_activation_tables
    bacc.get_activation_tables = _patched_act_tables(orig_tables)
    try:
        return _build_nc_inner(bacc, mybir, tile, f32, f32r, bf16, EXP, LN)
    finally:
        bacc.get_activation_tables = orig_tables


def _build_nc_inner(bacc, mybir, tile, f32, f32r, bf16, EXP, LN):
    f16 = mybir.dt.float16
    nc = bacc.Bacc(None, target_bir_lowering=False)

    xb_e = nc.declare_dram_parameter("xb", [C, H * W], f32r, isOutput=False)
    xchr_e = nc.declare_dram_parameter(
        "xchr", [128, NCHUNK * W], f32r, isOutput=False
    )
    gm_e = nc.declare_dram_parameter(
        "gmat", [128, NCHUNK * 128], f32r, isOutput=False
    )
    wp_e = nc.declare_dram_parameter("wpack", [C, 136], f32r, isOutput=False)
    xb16_e = nc.declare_dram_parameter("xb16", [C, H * W], f16, isOutput=False)
    wv16_e = nc.declare_dram_parameter("wv16", [C, C], f16, isOutput=False)
    out_e = nc.declare_dram_parameter("out", [C, NQ], f32, isOutput=True)

    hA = slice(0, 512)
    hB = slice(512, 1024)

    with tile.TileContext(nc) as tc:
        with (
            nc.allow_low_precision(
                reason="f32r/bf16 matmul operands; PSUM accumulates in f32"
            ),
            tc.tile_pool(name="const", bufs=1) as cst,
            tc.tile_pool(name="sb", bufs=1) as sbp,
            tc.tile_pool(name="pexp", bufs=3) as pexp,
        ):
            # ---- input DMAs.  Each queue pays ~2.4us startup latency on
            # its first transfer then streams at ~200GB/s, so xb halves
            # lead the scalar/gpsimd queues and everything lands ~11us.
            xb = cst.tile([C, H * W], f32r)
            nc.scalar.dma_start(xb[:, hA], xb_e[:, hA])
            nc.gpsimd.dma_start(xb[:, hB], xb_e[:, hB])
            wpack = cst.tile([C, 136], f32r)
            nc.sync.dma_start(wpack[:], wp_e[:])
            xchr = cst.tile([128, NCHUNK * W], f32r)
            nc.sync.dma_start(xchr[:], xchr_e[:])
            wv16 = cst.tile([C, C], f16)
            nc.sync.dma_start(wv16[:], wv16_e[:])
            xb16 = cst.tile([C, H * W], f16)
            nc.sync.dma_start(xb16[:], xb16_e[:])
            gm = cst.tile([128, NCHUNK * 128], f32r)
            nc.sync.dma_start(gm[:, 4 * 128 : 5 * 128], gm_e[:, 4 * 128 : 5 * 128])
            nc.scalar.dma_start(gm[:, 0 : 128], gm_e[:, 0 : 128])
            nc.sync.dma_start(gm[:, 128 : 2 * 128], gm_e[:, 128 : 2 * 128])
            nc.gpsimd.dma_start(gm[:, 2 * 128 : 4 * 128], gm_e[:, 2 * 128 : 4 * 128])
            wk = wpack[:, 0:D]
            wv = wpack[:, D : D + C]
            awT = wpack[0:W, D + C : D + C + 64]

            # ---- small constants (memset f32, cast to f32r) ----
            cf = cst.tile([D, 1], f32)
            nc.vector.memset(cf[:], 1.0)
            ones8 = cst.tile([D, 1], f32r)
            nc.vector.tensor_copy(ones8[:], cf[:])
            e65f = cst.tile([1, C + 1], f32)
            nc.vector.memset(e65f[:], 0.0)
            nc.vector.memset(e65f[:, C : C + 1], 3072.0)
            e65 = cst.tile([1, C + 1], f32r)
            nc.vector.tensor_copy(e65[:], e65f[:])
            zb = cst.tile([1, 1], f32)
            nc.vector.memset(zb[:], 0.0)
            # constant bias -20 on every score exp; see module docstring
            mB = cst.tile([128, 1], f32)
            nc.vector.memset(mB[:], -20.0)

            # ---- persistent SBUF tiles ----
            k_sb = sbp.tile([D, NK], f16)
            qfT = sbp.tile([D, NQ], f32r)   # f32r: S-row summation only
            qfT16A = sbp.tile([D, 512], f16)  # fp16: score matmul rhs
            qfT16B = sbp.tile([D, 512], f16)
            dex = sbp.tile([1, NQ], f32r)
            s_cl = sbp.tile([1, NQ], f32)
            vTa = sbp.tile([128, 8 * (C + 1)], bf16)
            nc.vector.memset(
                vTa[:].rearrange("p (t c) -> p t c", c=C + 1)[:, :, C], 1.0
            )

            with tc.tile_pool(name="ps_o", bufs=1, space="PSUM") as pso:
                out_ps = pso.tile([C + 1, NQ], f32)

                # ---- setup matmuls (all f32r) ----
                with tc.tile_pool(name="ps_s", bufs=1, space="PSUM") as pss:
                    # k projection first: only needs xb + wpack.  One
                    # PSUM bank, halves serialized on the copy (off the
                    # critical path); v tiles (double-buffered) fill PE
                    # while the kron operands stream in.
                    def vt_tile(u):
                        vt_ps = pss.tile(
                            [128, C], f32, tag="vtA" if u % 2 == 0 else "vtB"
                        )
                        nc.tensor.matmul(
                            vt_ps[:],
                            xb16[:, u * 128 : (u + 1) * 128],
                            wv16[:],
                            start=True,
                            stop=True,
                        )
                        if u % 2 == 0:
                            nc.vector.tensor_copy(
                                vTa[:, u * (C + 1) : u * (C + 1) + C],
                                vt_ps[:],
                            )
                        else:
                            nc.scalar.copy(
                                vTa[:, u * (C + 1) : u * (C + 1) + C],
                                vt_ps[:],
                            )

                    k_psA = pss.tile([D, 512], f32, tag="kps")
                    nc.tensor.matmul(
                        k_psA[:], wk, xb[:, hA], start=True, stop=True
                    )
                    nc.vector.tensor_copy(k_sb[:, hA], k_psA[:])
                    for u in range(3):
                        vt_tile(u)
                    k_psB = pss.tile([D, 512], f32, tag="kps")
                    nc.tensor.matmul(
                        k_psB[:], wk, xb[:, hB], start=True, stop=True
                    )
                    nc.scalar.copy(k_sb[:, hB], k_psB[:])

                    # fused q-proj + row-interp (kron), then col-interp
                    t2_ps = pss.tile([W, 128], f32, tag="t2")
                    korder = (4, 0, 1, 2, 3)
                    for j, k in enumerate(korder):
                        nc.tensor.matmul(
                            t2_ps[:],
                            xchr[:, k * W : (k + 1) * W],
                            gm[:, k * 128 : (k + 1) * 128],
                            start=(j == 0),
                            stop=(j == NCHUNK - 1),
                            skip_group_check=True,
                        )
                    t2_sb = sbp.tile([W, 128], f32r)
                    nc.scalar.copy(t2_sb[:], t2_ps[:])
                    for u in range(3, 8):
                        vt_tile(u)

                    # col-interp in two independent banks so each half's
                    # PSUM->SBUF copy starts as soon as its 8 matmuls end
                    qf_psA = pss.tile([D, 512], f32, tag="qfA")
                    qf_psB = pss.tile([D, 512], f32, tag="qfB")
                    t2_v = t2_sb[:].rearrange("c (i d) -> c i d", i=16)
                    for i in range(16):
                        dst = qf_psA if i < 8 else qf_psB
                        nc.tensor.matmul(
                            dst[:, (i % 8) * 64 : (i % 8) * 64 + 64],
                            t2_v[:, i, :],
                            awT,
                            start=True,
                            stop=True,
                        )
                    nc.vector.tensor_copy(qfT16A[:], qf_psA[:])
                    nc.scalar.copy(qfT16B[:], qf_psB[:])
                    nc.vector.tensor_copy(qfT[:, hA], qf_psA[:])
                    nc.scalar.copy(qfT[:, hB], qf_psB[:])


                # ---- main loop: scores^T, exp, accumulate ----
                with tc.tile_pool(
                    name="ps_sc", bufs=2, space="PSUM"
                ) as pssc:
                    for t in range(8):
                        sT = pssc.tile([128, NQ], f32)
                        kT_t = k_sb[:, t * 128 : (t + 1) * 128]
                        for h, qf16 in ((hA, qfT16A), (hB, qfT16B)):
                            nc.tensor.matmul(
                                sT[:, h], kT_t, qf16[:], start=True,
                                stop=True,
                            )
                        if t == 0:
                            # S = colsum(qfT) + clamp, in the exp0 shadow
                            # (PE idles waiting for pT otherwise).  Summing
                            # the same f32r qfT the scores use keeps the
                            # sign of borderline-|S| queries aligned with
                            # the reference's masked-row selection.
                            for h in (hA, hB):
                                s_ps = pssc.tile([1, 512], f32, tag="srow")
                                nc.tensor.matmul(
                                    s_ps[:], ones8[:], qfT[:, h],
                                    start=True, stop=True,
                                )
                                nc.vector.tensor_scalar(
                                    s_cl[:, h], s_ps[:], -4.65e-7, None,
                                    mybir.AluOpType.max,
                                )
                        pT = pexp.tile([128, NQ], bf16, tag="pT")
                        if t < 7:
                            nc.scalar.activation(
                                pT[:], sT[:], EXP, bias=mB[:]
                            )
                        else:
                            # halved so out7A (stop) lands earlier and the
                            # A-half epilogue overlaps the B-half tail
                            nc.scalar.activation(
                                pT[:, hA], sT[:, hA], EXP, bias=mB[:]
                            )
                            nc.scalar.activation(
                                pT[:, hB], sT[:, hB], EXP, bias=mB[:]
                            )
                        if t == 5:
                            # masked-key denominator term, off the
                            # critical path
                            nc.scalar.activation(
                                dex[:], s_cl[:], EXP, bias=mB[0:1, :],
                                scale=-1.0e8,
                            )
                        for h in (hA, hB):
                            nc.tensor.matmul(
                                out_ps[:, h],
                                vTa[:, t * (C + 1) : (t + 1) * (C + 1)],
                                pT[:, h],
                                start=(t == 0),
                                stop=(t == 7),
                                skip_group_check=True,
                            )
                        if t == 5:
                            for h in (hA, hB):
                                nc.tensor.matmul(
                                    out_ps[:, h],
                                    e65[:],
                                    dex[:, h],
                                    start=False,
                                    stop=False,
                                    skip_group_check=True,
                                )

                # ---- epilogue: rden = exp(-ln(den)) on ACT in halves
                # (ACT per-op overhead is ~270ns), broadcast on GPSIMD
                # and multiply on DVE in quarters for pipelining.  Reads
                # of out_ps serialize pairwise across engines, so the ln
                # halves and the mult quarters are emitted grouped. ----
                den = out_ps[C : C + 1, :]
                ldenA = sbp.tile([1, 512], f32, tag="ldA")
                rdenA = sbp.tile([1, 512], f32, tag="rdA")
                ldenB = sbp.tile([1, 512], f32, tag="ldB")
                rdenB = sbp.tile([1, 512], f32, tag="rdB")
                nc.scalar.activation(ldenA[:], den[:, hA], LN, bias=zb[:])
                nc.scalar.activation(
                    rdenA[:], ldenA[:], EXP, bias=zb[:], scale=-1.0
                )
                nc.scalar.activation(ldenB[:], den[:, hB], LN, bias=zb[:])
                nc.scalar.activation(
                    rdenB[:], ldenB[:], EXP, bias=zb[:], scale=-1.0
                )
                bcA = sbp.tile([C, 512], f32, tag="bcA")
                nc.gpsimd.partition_broadcast(bcA[:], rdenA[:])
                bcB = sbp.tile([C, 512], f32, tag="bcB")
                nc.gpsimd.partition_broadcast(bcB[:], rdenB[:])
                bcs = (bcA, bcA, bcB, bcB)
                fins = []
                for qi in range(4):
                    q = slice(qi * 256, (qi + 1) * 256)
                    fin = sbp.tile([C, 256], f32, tag=f"fin{qi}")
                    nc.vector.tensor_mul(
                        fin[:], out_ps[0:C, q],
                        bcs[qi][:, (qi % 2) * 256 : (qi % 2) * 256 + 256],
                    )
                    fins.append(fin)
                for qi, eng in zip(range(4), (nc.sync, nc.sync, nc.scalar, nc.sync)):
                    q = slice(qi * 256, (qi + 1) * 256)
                    eng.dma_start(out_e[:, q], fins[qi][:])

    nc.finalize()
    return nc


_NC = None


def _get_nc():
    global _NC
    if _NC is None:
        _NC = _build_nc()
    return _NC


def _in_maps(x, Wq, Wk, Wv):
    x = np.asarray(x, np.float32)
    Wq = np.asarray(Wq, np.float32)
    Wk = np.asarray(Wk, np.float32)
    Wv = np.asarray(Wv, np.float32)
    Ah = _lin_interp_mat(H, 2 * H)
    awT = np.ascontiguousarray(_lin_interp_mat(W, 2 * W).T)  # [32, 64]
    wpack = np.zeros((C, 136), np.float32)
    wpack[:, 0:D] = Wk.T
    wpack[:, D : D + C] = Wv.T
    wpack[0:W, D + C : D + C + 64] = awT
    # G_s[(c rloc), (i, d)] = Wq[d, c] * Ah_s[i, r0+rloc], c-major flat
    # row index (c*RWIN + rloc) split into NCHUNK chunks of 128
    gms = []
    for s in range(4):
        r0 = R_START[s]
        Ah_s = Ah[s * 16 : (s + 1) * 16, r0 : r0 + RWIN]  # [16, RWIN]
        G = np.einsum("dc,ir->crid", Wq, Ah_s).reshape(C * RWIN, 16 * D)
        gms.append(
            np.ascontiguousarray(
                G.reshape(NCHUNK, 128, 128)
                .transpose(1, 0, 2)
                .reshape(128, NCHUNK * 128)
            )
        )
    maps = []
    for i in range(N_CORES):
        b, s = divmod(i, 4)
        r0 = R_START[s]
        xb = np.ascontiguousarray(x[b].reshape(C, H * W))
        xsub = x[b][:, r0 : r0 + RWIN, :]  # [C, RWIN, W]
        xchr = np.ascontiguousarray(
            xsub.reshape(NCHUNK, 128, W).transpose(1, 0, 2).reshape(128, -1)
        )
        maps.append(
            {
                "xb": xb,
                "xb16": xb.astype(np.float16),
                "xchr": xchr,
                "gmat": gms[s],
                "wpack": wpack,
                "wv16": wpack[:, D : D + C].astype(np.float16),
            }
        )
    return maps


def _run(x, Wq, Wk, Wv, trace=False):
    from concourse.bass_utils import run_bass_kernel_spmd

    nc = _get_nc()
    res = run_bass_kernel_spmd(
        nc, _in_maps(x, Wq, Wk, Wv), core_ids=list(range(N_CORES)), trace=trace
    )
    out = np.empty((B, C, 4 * H * W), np.float32)
    for i in range(N_CORES):
        b, s = divmod(i, 4)
        out[b, :, s * NQ : (s + 1) * NQ] = res.results[i]["out"]
    return out.reshape(B, C, 2 * W, 2 * H), res


def kernel(x, Wq, Wk, Wv):
    out, _ = _run(x, Wq, Wk, Wv)
    return out


# revision 49
# speedup vs baseline: 1.3384x; 1.0257x over previous
"""Trainium2 Bass kernel for nn_AttentionConv (sparse checkerboard attention).

Math (per batch image, C=64, H=W=32, N=4096 upsampled tokens):
  q,k,v = 1x1 convs; q is bilinearly 2x-upsampled, k/v zero-upsampled
  (values only at (even,even) positions).  A checkerboard mask of -1e8 is
  added to k itself, so the 3072 masked key columns are all identically
  (-1e8,...,-1e8): their score for query n is -1e8*S(n) with
  S(n)=sum_d q_up[n,d], and their v is 0.  Hence
     out[c,n] = sum_{m' in 1024 unmasked} v[c,m'] exp(s[n,m']) / D(n)
     D(n)     = 3072*exp(-1e8*S(n)) + sum_{m'} exp(s[n,m'])
  with s[n,m'] = q_up[n,:].k[:,m'].

All exps carry a constant bias of -20 (out = num'/den' is invariant):
den' = den*e^-20 then spans [2e-6, 7e7], inside the domain where the ACT
ln table is accurate (it breaks past ~2^+-64), so 1/den can run on the
ACT engine as exp(-ln(den)) -- the DVE reciprocal is ~8 cycles/element
on a single lane and the ACT Reciprocal function is blocked in bass.

Device pipeline per core (matmul operands f32r; the big v^T.p
accumulation runs in bf16; PSUM accumulates in f32):
  k    = Wk x                                            [8, 1024]
  vT   = x^T Wv^T per 128-token chunk -> vTa (bf16, +ones denom row)
  t2   = kron(Wq, Ah_slice) contraction over the <=10 source rows the
         16-row slice touches (5 chunks of 128)          [32, 128]
  qfT  = col-interp of t2 via awT                        [8, 1024]
  dex  = exp(-1e8*max(S,-4.65e-7) - 20): masked-key denominator term;
         the clamp keeps the S<0 saturation finite (~1e15) because the
         exp table emits NaN for huge positive args
  loop over 8 key tiles: sT = k_t^T qfT -> exp -> bf16 pT ->
         out_ps[65,1024] += [v_t;1]^T pT
  rden = exp(-ln(den)), broadcast on GPSIMD, multiply on DVE, DMA out.

Sharding: 8 cores = 2 batches x 4 query-slices of 1024 tokens
(16 upsampled rows each).  No collectives; each core writes a disjoint
[64, 1024] output slice.
"""
import sys

import numpy as np

if "/opt/trn_rl_repo" not in sys.path:
    sys.path.insert(0, "/opt/trn_rl_repo")

B, C, H, W = 2, 64, 32, 32
D = 8          # q/k head dim
NQ = 1024      # query tokens per core (16 upsampled rows x 64 cols)
NK = 1024      # unmasked keys per image (= H*W)
N_CORES = 8
RWIN = 10      # source rows touched by one 16-row upsampled slice
NCHUNK = (C * RWIN) // 128  # = 5 kron contraction chunks
R_START = (0, 7, 15, 22)    # first source row per slice


def _lin_interp_mat(n_in, n_out):
    # float32 replica of reference's bilinear (align_corners=True) matrix
    pos = np.arange(n_out, dtype=np.float32) * np.float32(
        (n_in - 1) / (n_out - 1)
    )
    i0 = np.clip(np.floor(pos), 0, n_in - 2).astype(np.int32)
    w = (pos - i0.astype(np.float32)).astype(np.float32)
    A = np.zeros((n_out, n_in), np.float32)
    r = np.arange(n_out)
    np.add.at(A, (r, i0), 1.0 - w)
    np.add.at(A, (r, i0 + 1), w)
    return A


def _patched_act_tables(orig):
    """Pin Exp/Ln/Copy to the one table set that holds all of them
    (natural_log_exp_and_others).  The greedy table-load pass otherwise
    ping-pongs between an exp-only and an ln-only set in the epilogue,
    costing 4 extra 1.3us ACT_TABLE_LOADs on the critical path."""
    import functools

    @functools.cache
    def patched(arch):
        tabs = dict(orig(arch))
        combined = None
        for name, s in tabs.items():
            names = {f.name for f in s}
            if {"Exp", "Ln", "Copy"} <= names:
                combined = name
                break
        if combined is None:
            return tabs
        keep = tabs[combined]
        return {
            name: (s if name == combined else s - keep)
            for name, s in tabs.items()
        }

    return patched


def _build_nc():
    import concourse.bacc as bacc
    import concourse.mybir as mybir
    from concourse import tile

    f32 = mybir.dt.float32
    f32r = mybir.dt.float32r
    bf16 = mybir.dt.bfloat16
    EXP = mybir.ActivationFunctionType.Exp
    LN = mybir.ActivationFunctionType.Ln

    orig_tables = bacc.get_activation_tables
    bacc.get_activation_tables = _patched_act_tables(orig_tables)
    try:
        return _build_nc_inner(bacc, mybir, tile, f32, f32r, bf16, EXP, LN)
    finally:
        bacc.get_activation_tables = orig_tables


def _build_nc_inner(bacc, mybir, tile, f32, f32r, bf16, EXP, LN):
    f16 = mybir.dt.float16
    nc = bacc.Bacc(None, target_bir_lowering=False)

    xb_e = nc.declare_dram_parameter("xb", [C, H * W], f32r, isOutput=False)
    xchr_e = nc.declare_dram_parameter(
        "xchr", [128, NCHUNK * W], f32r, isOutput=False
    )
    gm_e = nc.declare_dram_parameter(
        "gmat", [128, NCHUNK * 128], f32r, isOutput=False
    )
    wp_e = nc.declare_dram_parameter("wpack", [C, 136], f32r, isOutput=False)
    xb16_e = nc.declare_dram_parameter("xb16", [C, H * W], f16, isOutput=False)
    wv16_e = nc.declare_dram_parameter("wv16", [C, C], f16, isOutput=False)
    out_e = nc.declare_dram_parameter("out", [C, NQ], f32, isOutput=True)

    hA = slice(0, 512)
    hB = slice(512, 1024)

    with tile.TileContext(nc) as tc:
        with (
            nc.allow_low_precision(
                reason="f32r/bf16 matmul operands; PSUM accumulates in f32"
            ),
            tc.tile_pool(name="const", bufs=1) as cst,
            tc.tile_pool(name="sb", bufs=1) as sbp,
            tc.tile_pool(name="pexp", bufs=3) as pexp,
        ):
            # ---- input DMAs.  Each queue pays ~2.4us startup latency on
            # its first transfer then streams at ~200GB/s, so xb halves
            # lead the scalar/gpsimd queues and everything lands ~11us.
            xb = cst.tile([C, H * W], f32r)
            nc.scalar.dma_start(xb[:, hA], xb_e[:, hA])
            nc.gpsimd.dma_start(xb[:, hB], xb_e[:, hB])
            wpack = cst.tile([C, 136], f32r)
            nc.sync.dma_start(wpack[:], wp_e[:])
            wv16 = cst.tile([C, C], f16)
            nc.sync.dma_start(wv16[:], wv16_e[:])
            xb16 = cst.tile([C, H * W], f16)
            nc.sync.dma_start(xb16[:], xb16_e[:])
            xchr = cst.tile([128, NCHUNK * W], f32r)
            nc.sync.dma_start(xchr[:], xchr_e[:])
            gm = cst.tile([128, NCHUNK * 128], f32r)
            nc.scalar.dma_start(gm[:, 0 : 128], gm_e[:, 0 : 128])
            nc.gpsimd.dma_start(gm[:, 2 * 128 : 4 * 128], gm_e[:, 2 * 128 : 4 * 128])
            nc.sync.dma_start(gm[:, 4 * 128 : 5 * 128], gm_e[:, 4 * 128 : 5 * 128])
            nc.sync.dma_start(gm[:, 128 : 2 * 128], gm_e[:, 128 : 2 * 128])
            wk = wpack[:, 0:D]
            wv = wpack[:, D : D + C]
            awT = wpack[0:W, D + C : D + C + 64]

            # ---- small constants (memset f32, cast to f32r) ----
            cf = cst.tile([D, 1], f32)
            nc.vector.memset(cf[:], 1.0)
            ones8 = cst.tile([D, 1], f32r)
            nc.vector.tensor_copy(ones8[:], cf[:])
            e65f = cst.tile([1, C + 1], f32)
            nc.vector.memset(e65f[:], 0.0)
            nc.vector.memset(e65f[:, C : C + 1], 3072.0)
            e65 = cst.tile([1, C + 1], f32r)
            nc.vector.tensor_copy(e65[:], e65f[:])
            zb = cst.tile([1, 1], f32)
            nc.vector.memset(zb[:], 0.0)
            # constant bias -20 on every score exp; see module docstring
            mB = cst.tile([128, 1], f32)
            nc.vector.memset(mB[:], -20.0)

            # ---- persistent SBUF tiles ----
            k_sb = sbp.tile([D, NK], f16)
            qfT = sbp.tile([D, NQ], f32r)   # f32r: S-row summation only
            qfT16A = sbp.tile([D, 512], f16)  # fp16: score matmul rhs
            qfT16B = sbp.tile([D, 512], f16)
            dex = sbp.tile([1, NQ], f32r)
            s_cl = sbp.tile([1, NQ], f32)
            vTa = sbp.tile([128, 8 * (C + 1)], bf16)
            nc.vector.memset(
                vTa[:].rearrange("p (t c) -> p t c", c=C + 1)[:, :, C], 1.0
            )

            with tc.tile_pool(name="ps_o", bufs=1, space="PSUM") as pso:
                out_ps = pso.tile([C + 1, NQ], f32)

                # ---- setup matmuls (all f32r) ----
                with tc.tile_pool(name="ps_s", bufs=1, space="PSUM") as pss:
                    # k projection first: only needs xb + wpack.  One
                    # PSUM bank, halves serialized on the copy (off the
                    # critical path); v tiles (double-buffered) fill PE
                    # while the kron operands stream in.
                    def vt_tile(u):
                        vt_ps = pss.tile(
                            [128, C], f32, tag="vtA" if u % 2 == 0 else "vtB"
                        )
                        nc.tensor.matmul(
                            vt_ps[:],
                            xb16[:, u * 128 : (u + 1) * 128],
                            wv16[:],
                            start=True,
                            stop=True,
                        )
                        if u % 2 == 0:
                            nc.vector.tensor_copy(
                                vTa[:, u * (C + 1) : u * (C + 1) + C],
                                vt_ps[:],
                            )
                        else:
                            nc.scalar.copy(
                                vTa[:, u * (C + 1) : u * (C + 1) + C],
                                vt_ps[:],
                            )

                    k_psA = pss.tile([D, 512], f32, tag="kps")
                    nc.tensor.matmul(
                        k_psA[:], wk, xb[:, hA], start=True, stop=True
                    )
                    nc.vector.tensor_copy(k_sb[:, hA], k_psA[:])

                    # fused q-proj + row-interp (kron), chunks in DMA
                    # arrival order
                    t2_ps = pss.tile([W, 128], f32, tag="t2")
                    korder = (0, 2, 3, 4, 1)
                    for j, k in enumerate(korder):
                        nc.tensor.matmul(
                            t2_ps[:],
                            xchr[:, k * W : (k + 1) * W],
                            gm[:, k * 128 : (k + 1) * 128],
                            start=(j == 0),
                            stop=(j == NCHUNK - 1),
                            skip_group_check=True,
                        )
                    t2_sb = sbp.tile([W, 128], f32r)
                    nc.scalar.copy(t2_sb[:], t2_ps[:])

                    k_psB = pss.tile([D, 512], f32, tag="kps")
                    nc.tensor.matmul(
                        k_psB[:], wk, xb[:, hB], start=True, stop=True
                    )
                    nc.scalar.copy(k_sb[:, hB], k_psB[:])
                    for u in range(8):
                        vt_tile(u)

                    # col-interp into two banks of the persistent pool:
                    # the loop's sT pool then opens without waiting for
                    # the f32r qf copies (pool-level WAR otherwise)
                    qf_psA = pso.tile([D, 512], f32, tag="qfA")
                    qf_psB = pso.tile([D, 512], f32, tag="qfB")
                    t2_v = t2_sb[:].rearrange("c (i d) -> c i d", i=16)
                    for i in range(16):
                        dst = qf_psA if i < 8 else qf_psB
                        nc.tensor.matmul(
                            dst[:, (i % 8) * 64 : (i % 8) * 64 + 64],
                            t2_v[:, i, :],
                            awT,
                            start=True,
                            stop=True,
                        )
                    nc.vector.tensor_copy(qfT16A[:], qf_psA[:])
                    nc.scalar.copy(qfT16B[:], qf_psB[:])
                    nc.vector.tensor_copy(qfT[:, hA], qf_psA[:])
                    nc.scalar.copy(qfT[:, hB], qf_psB[:])


                # ---- main loop: scores^T, exp, accumulate ----
                with tc.tile_pool(
                    name="ps_sc", bufs=2, space="PSUM"
                ) as pssc:
                    for t in range(8):
                        sT = pssc.tile([128, NQ], f32)
                        kT_t = k_sb[:, t * 128 : (t + 1) * 128]
                        for h, qf16 in ((hA, qfT16A), (hB, qfT16B)):
                            nc.tensor.matmul(
                                sT[:, h], kT_t, qf16[:], start=True,
                                stop=True,
                            )
                        if t == 0:
                            # S = colsum(qfT) + clamp, in the exp0 shadow
                            # (PE idles waiting for pT otherwise).  Summing
                            # the same f32r qfT the scores use keeps the
                            # sign of borderline-|S| queries aligned with
                            # the reference's masked-row selection.
                            for h, stag in ((hA, "qfA"), (hB, "qfB")):
                                s_ps = pso.tile([1, 512], f32, tag=stag)
                                nc.tensor.matmul(
                                    s_ps[:], ones8[:], qfT[:, h],
                                    start=True, stop=True,
                                )
                                nc.vector.tensor_scalar(
                                    s_cl[:, h], s_ps[:], -4.65e-7, None,
                                    mybir.AluOpType.max,
                                )
                        pT = pexp.tile([128, NQ], bf16, tag="pT")
                        if t < 7:
                            nc.scalar.activation(
                                pT[:], sT[:], EXP, bias=mB[:]
                            )
                        else:
                            # halved so out7A (stop) lands earlier and the
                            # A-half epilogue overlaps the B-half tail
                            nc.scalar.activation(
                                pT[:, hA], sT[:, hA], EXP, bias=mB[:]
                            )
                            nc.scalar.activation(
                                pT[:, hB], sT[:, hB], EXP, bias=mB[:]
                            )
                        if t == 5:
                            # masked-key denominator term, off the
                            # critical path
                            nc.scalar.activation(
                                dex[:], s_cl[:], EXP, bias=mB[0:1, :],
                                scale=-1.0e8,
                            )
                        for h in (hA, hB):
                            nc.tensor.matmul(
                                out_ps[:, h],
                                vTa[:, t * (C + 1) : (t + 1) * (C + 1)],
                                pT[:, h],
                                start=(t == 0),
                                stop=(t == 7),
                                skip_group_check=True,
                            )
                        if t == 5:
                            for h in (hA, hB):
                                nc.tensor.matmul(
                                    out_ps[:, h],
                                    e65[:],
                                    dex[:, h],
                                    start=False,
                                    stop=False,
                                    skip_group_check=True,
                                )

                # ---- epilogue: rden = exp(-ln(den)) on ACT in halves
                # (ACT per-op overhead is ~270ns), broadcast on GPSIMD
                # and multiply on DVE in quarters for pipelining.  Reads
                # of out_ps serialize pairwise across engines, so the ln
                # halves and the mult quarters are emitted grouped. ----
                den = out_ps[C : C + 1, :]
                ldenA = sbp.tile([1, 512], f32, tag="ldA")
                rdenA = sbp.tile([1, 512], f32, tag="rdA")
                ldenB = sbp.tile([1, 512], f32, tag="ldB")
                rdenB = sbp.tile([1, 512], f32, tag="rdB")
                nc.scalar.activation(ldenA[:], den[:, hA], LN, bias=zb[:])
                nc.scalar.activation(
                    rdenA[:], ldenA[:], EXP, bias=zb[:], scale=-1.0
                )
                nc.scalar.activation(ldenB[:], den[:, hB], LN, bias=zb[:])
                nc.scalar.activation(
                    rdenB[:], ldenB[:], EXP, bias=zb[:], scale=-1.0
                )
                bcA = sbp.tile([C, 512], f32, tag="bcA")
                nc.gpsimd.partition_broadcast(bcA[:], rdenA[:])
                bcB = sbp.tile([C, 512], f32, tag="bcB")
                nc.gpsimd.partition_broadcast(bcB[:], rdenB[:])
                bcs = (bcA, bcA, bcB, bcB)
                fins = []
                for qi in range(4):
                    q = slice(qi * 256, (qi + 1) * 256)
                    fin = sbp.tile([C, 256], f32, tag=f"fin{qi}")
                    nc.vector.tensor_mul(
                        fin[:], out_ps[0:C, q],
                        bcs[qi][:, (qi % 2) * 256 : (qi % 2) * 256 + 256],
                    )
                    fins.append(fin)
                for qi, eng in zip(range(4), (nc.sync, nc.sync, nc.scalar, nc.sync)):
                    q = slice(qi * 256, (qi + 1) * 256)
                    eng.dma_start(out_e[:, q], fins[qi][:])

    nc.finalize()
    return nc


_NC = None


def _get_nc():
    global _NC
    if _NC is None:
        _NC = _build_nc()
    return _NC


def _in_maps(x, Wq, Wk, Wv):
    x = np.asarray(x, np.float32)
    Wq = np.asarray(Wq, np.float32)
    Wk = np.asarray(Wk, np.float32)
    Wv = np.asarray(Wv, np.float32)
    Ah = _lin_interp_mat(H, 2 * H)
    awT = np.ascontiguousarray(_lin_interp_mat(W, 2 * W).T)  # [32, 64]
    wpack = np.zeros((C, 136), np.float32)
    wpack[:, 0:D] = Wk.T
    wpack[:, D : D + C] = Wv.T
    wpack[0:W, D + C : D + C + 64] = awT
    # G_s[(c rloc), (i, d)] = Wq[d, c] * Ah_s[i, r0+rloc], c-major flat
    # row index (c*RWIN + rloc) split into NCHUNK chunks of 128
    gms = []
    for s in range(4):
        r0 = R_START[s]
        Ah_s = Ah[s * 16 : (s + 1) * 16, r0 : r0 + RWIN]  # [16, RWIN]
        G = np.einsum("dc,ir->crid", Wq, Ah_s).reshape(C * RWIN, 16 * D)
        gms.append(
            np.ascontiguousarray(
                G.reshape(NCHUNK, 128, 128)
                .transpose(1, 0, 2)
                .reshape(128, NCHUNK * 128)
            )
        )
    maps = []
    for i in range(N_CORES):
        b, s = divmod(i, 4)
        r0 = R_START[s]
        xb = np.ascontiguousarray(x[b].reshape(C, H * W))
        xsub = x[b][:, r0 : r0 + RWIN, :]  # [C, RWIN, W]
        xchr = np.ascontiguousarray(
            xsub.reshape(NCHUNK, 128, W).transpose(1, 0, 2).reshape(128, -1)
        )
        maps.append(
            {
                "xb": xb,
                "xb16": xb.astype(np.float16),
                "xchr": xchr,
                "gmat": gms[s],
                "wpack": wpack,
                "wv16": wpack[:, D : D + C].astype(np.float16),
            }
        )
    return maps


def _run(x, Wq, Wk, Wv, trace=False):
    from concourse.bass_utils import run_bass_kernel_spmd

    nc = _get_nc()
    res = run_bass_kernel_spmd(
        nc, _in_maps(x, Wq, Wk, Wv), core_ids=list(range(N_CORES)), trace=trace
    )
    out = np.empty((B, C, 4 * H * W), np.float32)
    for i in range(N_CORES):
        b, s = divmod(i, 4)
        out[b, :, s * NQ : (s + 1) * NQ] = res.results[i]["out"]
    return out.reshape(B, C, 2 * W, 2 * H), res


def kernel(x, Wq, Wk, Wv):
    out, _ = _run(x, Wq, Wk, Wv)
    return out
